# revision 1
# baseline (speedup 1.0000x reference)
"""Trainium2 Bass kernel for windowed (Linformer-style) attention.

Reference computation (per input x in {x1, x2}):
  - split n=6400 into 8 windows of 800; per (batch, window, head): full attention
    with dim_head=16, then concat heads, project with Wout, add bias.
  - final output = path(x1) + path(x2).

Sharding: 2 inputs x 4 batches x 8 windows = 64 independent attention "block-inputs".
Core c gets (b,w) blocks g in {4c..4c+3}, both the x1 and x2 paths (8 block-inputs),
so the x1+x2 sum and output projection happen on-core.

v3 design notes (ACT-exp is the bottleneck engine; everything else hides
under it):
  - All attention matmuls in bf16 (1 cycle/row on the PE vs fp32's 4):
    host ships X^T pre-transposed in bf16; Q^T/K^T/V_aug/E tiles are bf16.
    Accumulation stays fp32 in PSUM; softmax denominator / reciprocal /
    normalization / output projection stay fp32.
  - Globally software-pipelined emission: the strict-FIFO PE queue gets
    S(t+1) BEFORE O(t), so while O(t) waits on exp(t), S(t+1) has already
    run and ACT can start exp(t+1) back-to-back.  QKV / V / xt-DMA of the
    next block-input are hoisted into the exp shadow of the current one,
    and the pair normalization + output projection are injected into the
    first steps of the following unit.
  - V augmented with a ones column (ones row in X^T + WVA layout) so the
    softmax denominator falls out of the O^T = V_aug^T @ E accumulation.
  - Per-head S^T / O^T matmuls use tile_position row/col tiling (4x32
    strips) so heads run concurrently in the PE array on hardware.
  - PSUM budget (8 banks): ps_s 2bufs x [128,1024] (4) for S^T;
    ps_ot 1buf x [128,1024] (2) shared by the O^T accumulator and the
    reciprocal broadcast; ps_p 2bufs x [128,512] (2) for projections.
"""

import os
import sys

sys.path.insert(0, "/opt/trn_rl_repo")

import numpy as np
import ml_dtypes
from contextlib import ExitStack

import concourse.bass as bass
import concourse.mybir as mybir
import concourse.tile as tile
from concourse import bacc
from concourse.bass_utils import run_bass_kernel_spmd

F32 = mybir.dt.float32
BF16 = mybir.dt.bfloat16
AF = mybir.ActivationFunctionType

DIM = 48
HEADS = 4
DH = 16
WS = 800        # window (i dimension)
WSP = 896       # j dimension padded to 7*128
INNER = 64
SCALE = DH ** -0.5  # 0.25
B = 4
N = 6400
NW = 8
NCORES = 8
BLOCKS_PER_CORE = 4
NBI = 8  # block-inputs per core: 4 blocks x 2 inputs
NJ = 7   # j-chunks per window: 7x128 (padded)
NSTEP = NJ * HEADS  # 28 (jc, h) steps per block-input
ICHUNKS = [(0, 512), (512, 800)]    # i (token) chunks, <=512 for fp32 matmul
KCHUNKS = [(0, 512), (512, 896)]    # K^T free-dim chunks

# unit order: pair p = block-inputs (p, p+4) -> process adjacently
UNITS = [0, 4, 1, 5, 2, 6, 3, 7]

USE_TILE_POSITION = os.environ.get("KERNEL_NO_TILEPOS", "0") != "1"
# benchmarking: repeat the whole kernel body R times inside the NEFF so the
# marginal wall-clock per rep isolates device time from dispatch overhead
REPS = int(os.environ.get("KERNEL_REPS", "1"))
# DIAG="noexp": replace the ACT exp with nothing (O reads a constant tile)
# to measure the PE/DVE/DMA pipeline floor. Diagnostic only — wrong results.
DIAG = os.environ.get("KERNEL_DIAG", "")


def _tp(row, col):
    return (row, col) if USE_TILE_POSITION else None


class _Body:
    """Emission helper holding pools/constants for one kernel body."""

    def __init__(self, ctx, tc, y_ap, xt_ap, consts):
        self.nc = tc.nc
        self.tc = tc
        self.y_ap = y_ap
        self.xt_ap = xt_ap
        (self.wq_s, self.wk_s, self.wva_s, self.wouta_s,
         self.b2_s) = consts

        self.xt_pool = ctx.enter_context(tc.tile_pool(name="xt", bufs=2))
        self.qkt_pool = ctx.enter_context(tc.tile_pool(name="qkt", bufs=2))
        self.vaug_pool = ctx.enter_context(tc.tile_pool(name="vaug", bufs=3))
        self.e_pool = ctx.enter_context(tc.tile_pool(name="e", bufs=4))
        self.ots_pool = ctx.enter_context(tc.tile_pool(name="ots", bufs=3))
        self.den_pool = ctx.enter_context(tc.tile_pool(name="den", bufs=2))
        self.ont_pool = ctx.enter_context(tc.tile_pool(name="ont", bufs=3))
        self.rs_pool = ctx.enter_context(tc.tile_pool(name="rs", bufs=2))
        self.outb_pool = ctx.enter_context(tc.tile_pool(name="outb", bufs=2))

        self.ps_s = ctx.enter_context(
            tc.tile_pool(name="ps_s", bufs=2, space="PSUM"))
        self.ps_ot = ctx.enter_context(
            tc.tile_pool(name="ps_ot", bufs=1, space="PSUM"))
        self.ps_p = ctx.enter_context(
            tc.tile_pool(name="ps_p", bufs=2, space="PSUM"))

        # per-unit live state
        self.xts = {}     # bi -> xt tile
        self.qkt = {}     # bi -> qkt tile
        self.va = {}      # (bi, jc) -> V_aug tile
        self.sg = {}      # (bi, t) -> S^T psum tile
        self.ot = {}      # bi -> O^T psum accumulator
        self.ots = {}     # bi -> O^T sbuf copy
        self.den = {}     # pair -> denominator tile
        self.rcp = {}     # pair -> reciprocal tile
        self.onts = {}    # pair -> [ont0, ont1]
        self.e_const = None

    # ---- emission pieces ----
    def emit_xt_dma(self, bi):
        nc = self.nc
        xts = self.xt_pool.tile([49, WSP], BF16, tag="xt", name="xts")
        nc.sync.dma_start(xts[:], self.xt_ap[bi, :, :])
        self.xts[bi] = xts

    def emit_qkv(self, bi, which):
        """which: 0 emits Q^T projection, 1 emits K^T projection."""
        nc = self.nc
        if bi not in self.qkt:
            self.qkt[bi] = self.qkt_pool.tile([128, 2, WSP], BF16, tag="qkt", name="qkt")
        qkt = self.qkt[bi]
        xts = self.xts[bi]
        w_s = self.wq_s if which == 0 else self.wk_s
        chunks = ICHUNKS if which == 0 else KCHUNKS
        for n0, n1 in chunks:
            pp = self.ps_p.tile([128, 512], F32, tag="p", name="pp")
            nc.tensor.matmul(pp[:, 0:n1 - n0], w_s[:, :], xts[0:48, n0:n1],
                             start=True, stop=True)
            nc.vector.tensor_copy(qkt[:, which, n0:n1], pp[:, 0:n1 - n0])

    def emit_v(self, bi, jc):
        nc = self.nc
        j0 = jc * 128
        va = self.vaug_pool.tile([128, 128], BF16, tag="va", name="va")
        vp = self.ps_p.tile([128, 512], F32, tag="p", name="pp")
        nc.tensor.matmul(vp[:, 0:128], self.xts[bi][:, j0:j0 + 128],
                         self.wva_s[:, :], start=True, stop=True)
        nc.vector.tensor_copy(va[:, :], vp[:, 0:128])
        self.va[(bi, jc)] = va

    def emit_s(self, bi, t):
        nc = self.nc
        jc, h = t // HEADS, t % HEADS
        j0 = jc * 128
        qkt = self.qkt[bi]
        sg = self.ps_s.tile([128, 1024], F32, tag="sg", name="sg")
        for n0, n1 in ICHUNKS:
            nc.tensor.matmul(
                sg[:, n0:n1],
                qkt[32 * h:32 * h + 16, 1, j0:j0 + 128],
                qkt[32 * h:32 * h + 16, 0, n0:n1],
                start=True, stop=True,
                tile_position=_tp(32 * h, 0),
            )
        self.sg[(bi, t)] = sg

    def emit_exp(self, bi, t):
        nc = self.nc
        sg = self.sg.pop((bi, t))
        if DIAG == "noexp":
            if self.e_const is None:
                self.e_const = self.e_pool.tile([128, WS], BF16, tag="ec",
                                                name="ec")
                nc.vector.memset(self.e_const[:], 1.0)
            return self.e_const
        e = self.e_pool.tile([128, WS], BF16, tag="e", name="e")
        nc.scalar.activation(e[:, :], sg[:, 0:WS], AF.Exp, scale=float(SCALE))
        return e

    def emit_o(self, bi, t, e):
        nc = self.nc
        jc, h = t // HEADS, t % HEADS
        if bi not in self.ot:
            self.ot[bi] = self.ps_ot.tile([128, 1024], F32, tag="ot", name="ot")
        ot = self.ot[bi]
        va = self.va[(bi, jc)]
        for n0, n1 in ICHUNKS:
            nc.tensor.matmul(
                ot[32 * h:32 * h + 32, n0:n1],
                va[:, 32 * h:32 * h + 32],
                e[:, n0:n1],
                start=(jc == 0), stop=(jc == NJ - 1),
                tile_position=_tp(0, 32 * h),
            )
        if h == HEADS - 1:
            del self.va[(bi, jc)]

    def emit_epilogue(self, bi, half, pair):
        """PSUM -> SBUF copy of O^T, denominator row gather."""
        nc = self.nc
        ot = self.ot.pop(bi)
        self.qkt.pop(bi, None)
        self.xts.pop(bi, None)
        ots = self.ots_pool.tile([128, WS], F32, tag="ots", name="ots")
        nc.vector.tensor_copy(ots[:], ot[:, 0:WS])
        self.ots[bi] = ots
        if pair not in self.den:
            self.den[pair] = self.den_pool.tile([8, WS], F32, tag="den", name="den")
        den = self.den[pair]
        for h in range(HEADS):
            nc.sync.dma_start(den[4 * half + h:4 * half + h + 1, :],
                              ots[32 * h + 16:32 * h + 17, :])

    def emit_recip(self, pair):
        """Start the denominator reciprocal (DVE) as early as possible."""
        nc = self.nc
        den = self.den.pop(pair)
        rcp = self.den_pool.tile([8, WS], F32, tag="rcp", name="rcp")
        scr = self.den_pool.tile([8, WS], F32, tag="scr", name="scr")
        nc.vector.reciprocal_approx_accurate(rcp[:], den[:], scr[:])
        rcpb = self.den_pool.tile([8, WS], BF16, tag="rcpb", name="rcpb")
        nc.vector.tensor_copy(rcpb[:], rcp[:])
        self.rcp[pair] = rcpb

    def emit_norm(self, pair):
        """Broadcast reciprocal rows to the O^T partition layout via a
        stride-0 DMA (keeps the PE out of it), then normalize on the DVE."""
        nc = self.nc
        rcp = self.rcp.pop(pair)
        onts = []
        for half in range(2):
            bi = pair + 4 * half
            rs = self.rs_pool.tile([128, WS], BF16, tag="rs", name="rs")
            src = rcp[4 * half:4 * half + 4, :].unsqueeze(1)
            nc.sync.dma_start(rs[:], src.to_broadcast([4, 32, WS]))
            ont = self.ont_pool.tile([128, WS], BF16, tag="ont", name="ont")
            nc.vector.tensor_mul(ont[:], self.ots.pop(bi)[:], rs[:])
            onts.append(ont)
        self.onts[pair] = onts

    def emit_outproj(self, pair):
        """out^T = Wout_all^T @ (ont0 + ont1): 4 matmuls sharing one LDW,
        bias via per-partition scalar add, transposed DMA to DRAM."""
        nc = self.nc
        onts = self.onts.pop(pair)
        outb = self.outb_pool.tile([48, WS], F32, tag="outb", name="outb")
        for n0, n1 in ICHUNKS:
            op = self.ps_p.tile([128, 512], F32, tag="p", name="pp")
            nc.tensor.matmul(op[0:48, 0:n1 - n0], self.wouta_s[:, :],
                             onts[0][:, n0:n1], start=True, stop=False)
            nc.tensor.matmul(op[0:48, 0:n1 - n0], self.wouta_s[:, :],
                             onts[1][:, n0:n1], start=False, stop=True)
            nc.vector.tensor_scalar_add(outb[:, n0:n1], op[0:48, 0:n1 - n0],
                                        self.b2_s[:, 0:1])
        nc.sync.dma_start(self.y_ap[pair].transpose([1, 0]), outb[:])


def build_kernel_body(ctx, tc, y_ap, xt_ap, wq_ap, wk_ap, wva_ap, wouta_ap,
                      b2_ap):
    nc = tc.nc
    consts = ctx.enter_context(tc.tile_pool(name="consts", bufs=1))
    wq_s = consts.tile([48, 128], BF16, tag="wq")
    nc.sync.dma_start(wq_s[:], wq_ap[:, :])
    wk_s = consts.tile([48, 128], BF16, tag="wk")
    nc.sync.dma_start(wk_s[:], wk_ap[:, :])
    wva_s = consts.tile([49, 128], BF16, tag="wva")
    nc.sync.dma_start(wva_s[:], wva_ap[:, :])
    wouta_s = consts.tile([128, 48], BF16, tag="wouta")
    nc.sync.dma_start(wouta_s[:], wouta_ap[:, :])
    b2_s = consts.tile([48, 1], F32, tag="b2")
    nc.sync.dma_start(b2_s[:], b2_ap[:, :])

    body = _Body(ctx, tc, y_ap, xt_ap,
                 (wq_s, wk_s, wva_s, wouta_s, b2_s))

    for _rep in range(REPS):
        _emit_pipeline(body)


def _emit_pipeline(body):
    """Flattened, software-pipelined emission over all 8 block-input units."""
    # prologue for the first unit
    bi0 = UNITS[0]
    body.emit_xt_dma(bi0)
    body.emit_qkv(bi0, 0)
    body.emit_qkv(bi0, 1)
    body.emit_v(bi0, 0)
    body.emit_s(bi0, 0)

    for u, bi in enumerate(UNITS):
        nxt = UNITS[u + 1] if u + 1 < len(UNITS) else None
        for t in range(NSTEP):
            # hoisted prologue work for the next unit, placed in the exp
            # shadow near the end of this unit
            if nxt is not None:
                if t == 20:
                    body.emit_xt_dma(nxt)
                elif t == 22:
                    body.emit_qkv(nxt, 0)
                elif t == 24:
                    body.emit_qkv(nxt, 1)
            # deferred normalization of the previous pair, injected early in
            # this unit so its PE/DVE work hides under this unit's exp stream
            if u % 2 == 0 and u > 0:
                if t == 2:
                    body.emit_norm(u // 2 - 1)
                elif t == 4:
                    body.emit_outproj(u // 2 - 1)
            # emit S one step ahead of O so the in-order PE queue never has
            # an un-issued S behind a stalled O
            if t + 1 < NSTEP:
                if (t + 1) % HEADS == 0:
                    body.emit_v(bi, (t + 1) // HEADS)
                body.emit_s(bi, t + 1)
            elif nxt is not None:
                body.emit_v(nxt, 0)
                body.emit_s(nxt, 0)
            e = body.emit_exp(bi, t)
            body.emit_o(bi, t, e)
        body.emit_epilogue(bi, half=u % 2, pair=u // 2)
        if u % 2 == 1:
            body.emit_recip(u // 2)

    # final pair's normalization runs in the tail
    last_pair = len(UNITS) // 2 - 1
    body.emit_norm(last_pair)
    body.emit_outproj(last_pair)


def _dedup_ldweights(nc):
    """Drop InstLdweights that reload the exact weights the PE already holds.

    tile_legalize splits every non-f32 matmul into LDW+MM; chunked matmuls
    sharing one stationary operand then carry a redundant second LDW (no
    waits/updates).  Removing it saves ~110ns of PE sequencer time each."""
    fn = nc.m.functions[0]
    for bb in fn.blocks:
        insts = bb.instructions
        drop = []
        prev_key = None
        for k in range(len(insts)):
            inst = insts[k]
            tn = type(inst).__name__
            if str(inst.engine) != "EngineType.PE":
                continue
            if tn == "InstLdweights":
                w = inst.ins[0]
                key = (w.memsetref, w.offset, str(w.ap), str(w.dtype),
                       str(inst.tile_position), str(inst.tile_size),
                       str(inst.perf_mode))
                si = inst.sync_info
                clean = si is None or (not si.on_wait and not si.on_update)
                if key == prev_key and clean:
                    drop.append(k)
                    continue
                prev_key = key
            elif tn == "InstMatmult":
                pass  # same weights keep streaming; array not clobbered
            else:
                prev_key = None
        for k in reversed(drop):
            del insts[k]


_CACHED = {}


def build_nc():
    key = (REPS, USE_TILE_POSITION, DIAG)
    if key in _CACHED:
        return _CACHED[key]
    nc = bacc.Bacc("TRN2", target_bir_lowering=False, debug=False)
    xt = nc.dram_tensor("xt", [NBI, 49, WSP], BF16, kind="ExternalInput")
    wq = nc.dram_tensor("wq", [48, 128], BF16, kind="ExternalInput")
    wk = nc.dram_tensor("wk", [48, 128], BF16, kind="ExternalInput")
    wva = nc.dram_tensor("wva", [49, 128], BF16, kind="ExternalInput")
    wouta = nc.dram_tensor("wouta", [128, 48], BF16, kind="ExternalInput")
    b2 = nc.dram_tensor("b2", [48, 1], F32, kind="ExternalInput")
    y = nc.dram_tensor("y", [BLOCKS_PER_CORE, WS, DIM], F32,
                       kind="ExternalOutput")
    with tile.TileContext(nc) as tc:
        with ExitStack() as ctx:
            build_kernel_body(ctx, tc, y.ap(), xt.ap(), wq.ap(), wk.ap(),
                              wva.ap(), wouta.ap(), b2.ap())
    nc.compile()
    _dedup_ldweights(nc)
    _CACHED[key] = nc
    return nc


def _prep_consts(Wqkv, Wout, bout):
    WQ = np.zeros((48, 128), np.float32)
    WK = np.zeros((48, 128), np.float32)
    WVA = np.zeros((49, 128), np.float32)
    WOUTA = np.zeros((128, 48), np.float32)
    for h in range(HEADS):
        WQ[:, 32 * h:32 * h + 16] = Wqkv[h * 48:h * 48 + 16, :].T
        WK[:, 32 * h:32 * h + 16] = Wqkv[h * 48 + 16:h * 48 + 32, :].T
        WVA[0:48, 32 * h:32 * h + 16] = Wqkv[h * 48 + 32:h * 48 + 48, :].T
        WVA[48, 32 * h + 16] = 1.0
        WOUTA[32 * h:32 * h + 16, :] = Wout[:, 16 * h:16 * h + 16].T
    B2 = (2.0 * bout).astype(np.float32)[:, None]
    BF = ml_dtypes.bfloat16
    return (WQ.astype(BF), WK.astype(BF), WVA.astype(BF), WOUTA.astype(BF),
            B2)


def kernel(x1, x2, Wqkv, Wout, bout):
    x1 = np.ascontiguousarray(x1, np.float32)
    x2 = np.ascontiguousarray(x2, np.float32)
    Wqkv = np.asarray(Wqkv, np.float32)
    Wout = np.asarray(Wout, np.float32)
    bout = np.asarray(bout, np.float32)

    nc = build_nc()
    WQ, WK, WVA, WOUTA, B2 = _prep_consts(Wqkv, Wout, bout)

    in_maps = []
    for c in range(NCORES):
        XT = np.zeros((NBI, 49, WSP), ml_dtypes.bfloat16)
        for j in range(BLOCKS_PER_CORE):
            g = 4 * c + j
            b, w = g // NW, g % NW
            XT[j, 0:48, 0:WS] = x1[b, w * WS:(w + 1) * WS, :].T
            XT[j, 48, 0:WS] = 1.0
            XT[4 + j, 0:48, 0:WS] = x2[b, w * WS:(w + 1) * WS, :].T
            XT[4 + j, 48, 0:WS] = 1.0
        in_maps.append({
            "xt": XT, "wq": WQ, "wk": WK, "wva": WVA,
            "wouta": WOUTA, "b2": B2,
        })

    res = run_bass_kernel_spmd(nc, in_maps, core_ids=list(range(NCORES)))
    kernel._last_results = res

    out = np.empty((B, N, DIM), np.float32)
    for c in range(NCORES):
        y = res.results[c]["y"]
        for j in range(BLOCKS_PER_CORE):
            g = 4 * c + j
            b, w = g // NW, g % NW
            out[b, w * WS:(w + 1) * WS, :] = y[j]
    return out



# revision 9
# speedup vs baseline: 1.4159x; 1.4159x over previous
"""Trainium2 Bass kernel for windowed (Linformer-style) attention.

Reference computation (per input x in {x1, x2}):
  - split n=6400 into 8 windows of 800; per (batch, window, head): full attention
    with dim_head=16, then concat heads, project with Wout, add bias.
  - final output = path(x1) + path(x2).

Sharding: 2 inputs x 4 batches x 8 windows = 64 independent attention "block-inputs".
Core c gets (b,w) blocks g in {4c..4c+3}, both the x1 and x2 paths (8 block-inputs),
so the x1+x2 sum and output projection happen on-core.

v3 design notes (ACT-exp is the bottleneck engine; everything else hides
under it):
  - All attention matmuls in bf16 (1 cycle/row on the PE vs fp32's 4):
    host ships X^T pre-transposed in bf16; Q^T/K^T/V_aug/E tiles are bf16.
    Accumulation stays fp32 in PSUM; softmax denominator / reciprocal /
    normalization / output projection stay fp32.
  - Globally software-pipelined emission: the strict-FIFO PE queue gets
    S(t+1) BEFORE O(t), so while O(t) waits on exp(t), S(t+1) has already
    run and ACT can start exp(t+1) back-to-back.  QKV / V / xt-DMA of the
    next block-input are hoisted into the exp shadow of the current one,
    and the pair normalization + output projection are injected into the
    first steps of the following unit.
  - V augmented with a ones column (ones row in X^T + WVA layout) so the
    softmax denominator falls out of the O^T = V_aug^T @ E accumulation.
  - Per-head S^T / O^T matmuls use tile_position row/col tiling (4x32
    strips) so heads run concurrently in the PE array on hardware.
  - PSUM budget (8 banks): ps_s 2bufs x [128,1024] (4) for S^T;
    ps_ot 1buf x [128,1024] (2) shared by the O^T accumulator and the
    reciprocal broadcast; ps_p 2bufs x [128,512] (2) for projections.
"""

import os
import sys

sys.path.insert(0, "/opt/trn_rl_repo")

import numpy as np
import ml_dtypes
from contextlib import ExitStack

import concourse.bass as bass
import concourse.mybir as mybir
import concourse.tile as tile
from concourse import bacc
from concourse.bass_utils import run_bass_kernel_spmd

F32 = mybir.dt.float32
BF16 = mybir.dt.bfloat16
AF = mybir.ActivationFunctionType

DIM = 48
HEADS = 4
DH = 16
WS = 800        # window (i dimension)
WSP = 896       # j dimension padded to 7*128
INNER = 64
SCALE = DH ** -0.5  # 0.25
B = 4
N = 6400
NW = 8
NCORES = 8
BLOCKS_PER_CORE = 4
NBI = 8  # block-inputs per core: 4 blocks x 2 inputs
NJ = 7   # j-chunks per window: 7x128 (padded)
NSTEP = NJ * HEADS  # 28 (jc, h) steps per block-input
ICHUNKS = [(0, 512), (512, 800)]    # i (token) chunks, <=512 for fp32 matmul
KCHUNKS = [(0, 512), (512, 896)]    # K^T free-dim chunks

# unit order: pair p = block-inputs (p, p+4) -> process adjacently
UNITS = [0, 4, 1, 5, 2, 6, 3, 7]

USE_TILE_POSITION = os.environ.get("KERNEL_NO_TILEPOS", "0") != "1"
# benchmarking: repeat the whole kernel body R times inside the NEFF so the
# marginal wall-clock per rep isolates device time from dispatch overhead
REPS = int(os.environ.get("KERNEL_REPS", "1"))
# DIAG="noexp": replace the ACT exp with nothing (O reads a constant tile)
# to measure the PE/DVE/DMA pipeline floor. Diagnostic only — wrong results.
DIAG = os.environ.get("KERNEL_DIAG", "")


def _tp(row, col):
    return (row, col) if USE_TILE_POSITION else None


class _Body:
    """Emission helper holding pools/constants for one kernel body."""

    def __init__(self, ctx, tc, y_ap, xt_ap, consts):
        self.nc = tc.nc
        self.tc = tc
        self.y_ap = y_ap
        self.xt_ap = xt_ap
        (self.wq_s, self.wk_s, self.wva_s, self.wouta_s,
         self.b2_s) = consts

        self.xt_pool = ctx.enter_context(tc.tile_pool(name="xt", bufs=2))
        self.qkt_pool = ctx.enter_context(tc.tile_pool(name="qkt", bufs=2))
        self.vaug_pool = ctx.enter_context(tc.tile_pool(name="vaug", bufs=3))
        self.e_pool = ctx.enter_context(tc.tile_pool(name="e", bufs=4))
        self.ots_pool = ctx.enter_context(tc.tile_pool(name="ots", bufs=3))
        self.den_pool = ctx.enter_context(tc.tile_pool(name="den", bufs=2))
        self.ont_pool = ctx.enter_context(tc.tile_pool(name="ont", bufs=3))
        self.rs_pool = ctx.enter_context(tc.tile_pool(name="rs", bufs=2))
        self.outb_pool = ctx.enter_context(tc.tile_pool(name="outb", bufs=2))

        self.ps_s = ctx.enter_context(
            tc.tile_pool(name="ps_s", bufs=2, space="PSUM"))
        self.ps_ot = ctx.enter_context(
            tc.tile_pool(name="ps_ot", bufs=1, space="PSUM"))
        self.ps_p = ctx.enter_context(
            tc.tile_pool(name="ps_p", bufs=2, space="PSUM"))

        # per-unit live state
        self.xts = {}     # bi -> xt tile
        self.qkt = {}     # bi -> qkt tile
        self.va = {}      # (bi, jc) -> V_aug tile
        self.sg = {}      # (bi, t) -> S^T psum tile
        self.ot = {}      # bi -> O^T psum accumulator
        self.ots = {}     # bi -> O^T sbuf copy
        self.den = {}     # pair -> denominator tile
        self.rcp = {}     # pair -> reciprocal tile
        self.onts = {}    # pair -> [ont0, ont1]
        self.e_const = None

    # ---- emission pieces ----
    def emit_xt_dma(self, bi):
        nc = self.nc
        xts = self.xt_pool.tile([49, WSP], BF16, tag="xt", name="xts")
        nc.sync.dma_start(xts[:], self.xt_ap[bi, :, :])
        self.xts[bi] = xts

    def emit_qkv(self, bi, which):
        """which: 0 emits Q^T projection, 1 emits K^T projection."""
        nc = self.nc
        if bi not in self.qkt:
            self.qkt[bi] = self.qkt_pool.tile([128, 2, WSP], BF16, tag="qkt", name="qkt")
        qkt = self.qkt[bi]
        xts = self.xts[bi]
        w_s = self.wq_s if which == 0 else self.wk_s
        chunks = ICHUNKS if which == 0 else KCHUNKS
        for n0, n1 in chunks:
            pp = self.ps_p.tile([128, 512], F32, tag="p", name="pp")
            nc.tensor.matmul(pp[:, 0:n1 - n0], w_s[:, :], xts[0:48, n0:n1],
                             start=True, stop=True)
            nc.vector.tensor_copy(qkt[:, which, n0:n1], pp[:, 0:n1 - n0])

    def emit_v(self, bi, jc):
        nc = self.nc
        j0 = jc * 128
        va = self.vaug_pool.tile([128, 128], BF16, tag="va", name="va")
        vp = self.ps_p.tile([128, 512], F32, tag="p", name="pp")
        nc.tensor.matmul(vp[:, 0:128], self.xts[bi][:, j0:j0 + 128],
                         self.wva_s[:, :], start=True, stop=True)
        nc.vector.tensor_copy(va[:, :], vp[:, 0:128])
        self.va[(bi, jc)] = va

    def emit_s(self, bi, t):
        nc = self.nc
        jc, h = t // HEADS, t % HEADS
        j0 = jc * 128
        qkt = self.qkt[bi]
        sg = self.ps_s.tile([128, 1024], F32, tag="sg", name="sg")
        for n0, n1 in ICHUNKS:
            nc.tensor.matmul(
                sg[:, n0:n1],
                qkt[32 * h:32 * h + 16, 1, j0:j0 + 128],
                qkt[32 * h:32 * h + 16, 0, n0:n1],
                start=True, stop=True,
                tile_position=_tp(32 * h, 0),
            )
        self.sg[(bi, t)] = sg

    def emit_exp(self, bi, t):
        nc = self.nc
        sg = self.sg.pop((bi, t))
        if DIAG == "noexp":
            if self.e_const is None:
                self.e_const = self.e_pool.tile([128, WS], BF16, tag="ec",
                                                name="ec")
                nc.vector.memset(self.e_const[:], 1.0)
            return self.e_const
        e = self.e_pool.tile([128, WS], BF16, tag="e", name="e")
        nc.scalar.activation(e[:, :], sg[:, 0:WS], AF.Exp, scale=float(SCALE))
        return e

    def emit_o(self, bi, t, e):
        nc = self.nc
        jc, h = t // HEADS, t % HEADS
        if bi not in self.ot:
            self.ot[bi] = self.ps_ot.tile([128, 1024], F32, tag="ot", name="ot")
        ot = self.ot[bi]
        va = self.va[(bi, jc)]
        for n0, n1 in ICHUNKS:
            nc.tensor.matmul(
                ot[32 * h:32 * h + 32, n0:n1],
                va[:, 32 * h:32 * h + 32],
                e[:, n0:n1],
                start=(jc == 0), stop=(jc == NJ - 1),
                tile_position=_tp(0, 32 * h),
            )
        if h == HEADS - 1:
            del self.va[(bi, jc)]

    def emit_epilogue(self, bi, half, pair):
        """PSUM -> SBUF copy of O^T, denominator row gather."""
        nc = self.nc
        ot = self.ot.pop(bi)
        self.qkt.pop(bi, None)
        self.xts.pop(bi, None)
        ots = self.ots_pool.tile([128, WS], F32, tag="ots", name="ots")
        nc.vector.tensor_copy(ots[:], ot[:, 0:WS])
        self.ots[bi] = ots
        if pair not in self.den:
            self.den[pair] = self.den_pool.tile([8, WS], F32, tag="den", name="den")
        den = self.den[pair]
        for h in range(HEADS):
            nc.sync.dma_start(den[4 * half + h:4 * half + h + 1, :],
                              ots[32 * h + 16:32 * h + 17, :])

    def emit_recip(self, pair):
        """Denominator reciprocal (DVE) + broadcast DMAs, as early as
        possible so the norm multiply and projection never wait on them."""
        nc = self.nc
        den = self.den.pop(pair)
        rcp = self.den_pool.tile([8, WS], F32, tag="rcp", name="rcp")
        scr = self.den_pool.tile([8, WS], F32, tag="scr", name="scr")
        nc.vector.reciprocal_approx_accurate(rcp[:], den[:], scr[:])
        rcpb = self.den_pool.tile([8, WS], BF16, tag="rcpb", name="rcpb")
        nc.vector.tensor_copy(rcpb[:], rcp[:])
        rss = []
        for half in range(2):
            rs = self.rs_pool.tile([128, WS], BF16, tag="rs", name="rs")
            src = rcpb[4 * half:4 * half + 4, :].unsqueeze(1)
            nc.sync.dma_start(rs[:], src.to_broadcast([4, 32, WS]))
            rss.append(rs)
        self.rcp[pair] = rss

    def emit_norm(self, pair):
        """Normalize on the DVE (the reciprocal broadcast DMAs were issued
        back at emit_recip, half a unit earlier)."""
        nc = self.nc
        rss = self.rcp.pop(pair)
        onts = []
        for half in range(2):
            bi = pair + 4 * half
            ont = self.ont_pool.tile([128, WS], BF16, tag="ont", name="ont")
            nc.vector.tensor_mul(ont[:], self.ots.pop(bi)[:], rss[half])
            onts.append(ont)
        self.onts[pair] = onts

    def emit_outproj(self, pair):
        """out^T = Wout_all^T @ (ont0 + ont1): 4 matmuls sharing one LDW,
        bias via per-partition scalar add, transposed DMA to DRAM."""
        nc = self.nc
        onts = self.onts.pop(pair)
        outb = self.outb_pool.tile([48, WS], F32, tag="outb", name="outb")
        for n0, n1 in ICHUNKS:
            op = self.ps_p.tile([128, 512], F32, tag="p", name="pp")
            nc.tensor.matmul(op[0:48, 0:n1 - n0], self.wouta_s[:, :],
                             onts[0][:, n0:n1], start=True, stop=False)
            nc.tensor.matmul(op[0:48, 0:n1 - n0], self.wouta_s[:, :],
                             onts[1][:, n0:n1], start=False, stop=True)
            nc.vector.tensor_scalar_add(outb[:, n0:n1], op[0:48, 0:n1 - n0],
                                        self.b2_s[:, 0:1])
        nc.sync.dma_start(self.y_ap[pair], outb[:])


def build_kernel_body(ctx, tc, y_ap, xt_ap, wq_ap, wk_ap, wva_ap, wouta_ap,
                      b2_ap):
    nc = tc.nc
    consts = ctx.enter_context(tc.tile_pool(name="consts", bufs=1))
    wq_s = consts.tile([48, 128], BF16, tag="wq")
    nc.sync.dma_start(wq_s[:], wq_ap[:, :])
    wk_s = consts.tile([48, 128], BF16, tag="wk")
    nc.sync.dma_start(wk_s[:], wk_ap[:, :])
    wva_s = consts.tile([49, 128], BF16, tag="wva")
    nc.sync.dma_start(wva_s[:], wva_ap[:, :])
    wouta_s = consts.tile([128, 48], BF16, tag="wouta")
    nc.sync.dma_start(wouta_s[:], wouta_ap[:, :])
    b2_s = consts.tile([48, 1], F32, tag="b2")
    nc.sync.dma_start(b2_s[:], b2_ap[:, :])

    body = _Body(ctx, tc, y_ap, xt_ap,
                 (wq_s, wk_s, wva_s, wouta_s, b2_s))

    for _rep in range(REPS):
        _emit_pipeline(body)


def _emit_pipeline(body):
    """Flattened, software-pipelined emission over all 8 block-input units."""
    # prologue for the first unit
    bi0 = UNITS[0]
    body.emit_xt_dma(bi0)
    body.emit_qkv(bi0, 0)
    body.emit_qkv(bi0, 1)
    body.emit_v(bi0, 0)
    body.emit_s(bi0, 0)

    for u, bi in enumerate(UNITS):
        nxt = UNITS[u + 1] if u + 1 < len(UNITS) else None
        for t in range(NSTEP):
            # hoisted prologue work for the next unit, placed in the exp
            # shadow near the end of this unit
            if nxt is not None:
                if t == 20:
                    body.emit_xt_dma(nxt)
                elif t == 22:
                    body.emit_qkv(nxt, 0)
                elif t == 24:
                    body.emit_qkv(nxt, 1)
            # deferred normalization of the previous pair, injected early in
            # this unit so its PE/DVE work hides under this unit's exp stream
            if u % 2 == 0 and u > 0:
                if t == 2:
                    body.emit_norm(u // 2 - 1)
                elif t == 8:
                    body.emit_outproj(u // 2 - 1)
            # emit S one step ahead of O so the in-order PE queue never has
            # an un-issued S behind a stalled O
            if t + 1 < NSTEP:
                if (t + 1) % HEADS == 0:
                    body.emit_v(bi, (t + 1) // HEADS)
                body.emit_s(bi, t + 1)
            elif nxt is not None:
                body.emit_v(nxt, 0)
                body.emit_s(nxt, 0)
            e = body.emit_exp(bi, t)
            body.emit_o(bi, t, e)
        body.emit_epilogue(bi, half=u % 2, pair=u // 2)
        if u % 2 == 1:
            body.emit_recip(u // 2)

    # final pair's normalization runs in the tail
    last_pair = len(UNITS) // 2 - 1
    body.emit_norm(last_pair)
    body.emit_outproj(last_pair)


def _dedup_ldweights(nc):
    """Drop InstLdweights that reload the exact weights the PE already holds.

    tile_legalize splits every non-f32 matmul into LDW+MM; chunked matmuls
    sharing one stationary operand then carry a redundant second LDW (no
    waits/updates).  Removing it saves ~110ns of PE sequencer time each."""
    fn = nc.m.functions[0]
    for bb in fn.blocks:
        insts = bb.instructions
        drop = []
        prev_key = None
        for k in range(len(insts)):
            inst = insts[k]
            tn = type(inst).__name__
            if str(inst.engine) != "EngineType.PE":
                continue
            if tn == "InstLdweights":
                w = inst.ins[0]
                key = (w.memsetref, w.offset, str(w.ap), str(w.dtype),
                       str(inst.tile_position), str(inst.tile_size),
                       str(inst.perf_mode))
                si = inst.sync_info
                clean = si is None or (not si.on_wait and not si.on_update)
                if key == prev_key and clean:
                    drop.append(k)
                    continue
                prev_key = key
            elif tn == "InstMatmult":
                pass  # same weights keep streaming; array not clobbered
            else:
                prev_key = None
        for k in reversed(drop):
            del insts[k]


_CACHED = {}


def build_nc():
    key = (REPS, USE_TILE_POSITION, DIAG)
    if key in _CACHED:
        return _CACHED[key]
    nc = bacc.Bacc("TRN2", target_bir_lowering=False, debug=False)
    xt = nc.dram_tensor("xt", [NBI, 49, WSP], BF16, kind="ExternalInput")
    wq = nc.dram_tensor("wq", [48, 128], BF16, kind="ExternalInput")
    wk = nc.dram_tensor("wk", [48, 128], BF16, kind="ExternalInput")
    wva = nc.dram_tensor("wva", [49, 128], BF16, kind="ExternalInput")
    wouta = nc.dram_tensor("wouta", [128, 48], BF16, kind="ExternalInput")
    b2 = nc.dram_tensor("b2", [48, 1], F32, kind="ExternalInput")
    y = nc.dram_tensor("y", [BLOCKS_PER_CORE, DIM, WS], F32,
                       kind="ExternalOutput")
    with tile.TileContext(nc) as tc:
        with ExitStack() as ctx:
            build_kernel_body(ctx, tc, y.ap(), xt.ap(), wq.ap(), wk.ap(),
                              wva.ap(), wouta.ap(), b2.ap())
    nc.compile()
    _dedup_ldweights(nc)
    _CACHED[key] = nc
    return nc


def _prep_consts(Wqkv, Wout, bout):
    WQ = np.zeros((48, 128), np.float32)
    WK = np.zeros((48, 128), np.float32)
    WVA = np.zeros((49, 128), np.float32)
    WOUTA = np.zeros((128, 48), np.float32)
    for h in range(HEADS):
        WQ[:, 32 * h:32 * h + 16] = Wqkv[h * 48:h * 48 + 16, :].T
        WK[:, 32 * h:32 * h + 16] = Wqkv[h * 48 + 16:h * 48 + 32, :].T
        WVA[0:48, 32 * h:32 * h + 16] = Wqkv[h * 48 + 32:h * 48 + 48, :].T
        WVA[48, 32 * h + 16] = 1.0
        WOUTA[32 * h:32 * h + 16, :] = Wout[:, 16 * h:16 * h + 16].T
    B2 = (2.0 * bout).astype(np.float32)[:, None]
    BF = ml_dtypes.bfloat16
    return (WQ.astype(BF), WK.astype(BF), WVA.astype(BF), WOUTA.astype(BF),
            B2)


def kernel(x1, x2, Wqkv, Wout, bout):
    x1 = np.ascontiguousarray(x1, np.float32)
    x2 = np.ascontiguousarray(x2, np.float32)
    Wqkv = np.asarray(Wqkv, np.float32)
    Wout = np.asarray(Wout, np.float32)
    bout = np.asarray(bout, np.float32)

    nc = build_nc()
    WQ, WK, WVA, WOUTA, B2 = _prep_consts(Wqkv, Wout, bout)

    in_maps = []
    for c in range(NCORES):
        XT = np.zeros((NBI, 49, WSP), ml_dtypes.bfloat16)
        for j in range(BLOCKS_PER_CORE):
            g = 4 * c + j
            b, w = g // NW, g % NW
            XT[j, 0:48, 0:WS] = x1[b, w * WS:(w + 1) * WS, :].T
            XT[j, 48, 0:WS] = 1.0
            XT[4 + j, 0:48, 0:WS] = x2[b, w * WS:(w + 1) * WS, :].T
            XT[4 + j, 48, 0:WS] = 1.0
        in_maps.append({
            "xt": XT, "wq": WQ, "wk": WK, "wva": WVA,
            "wouta": WOUTA, "b2": B2,
        })

    res = run_bass_kernel_spmd(nc, in_maps, core_ids=list(range(NCORES)))
    kernel._last_results = res

    out = np.empty((B, N, DIM), np.float32)
    for c in range(NCORES):
        y = res.results[c]["y"]
        for j in range(BLOCKS_PER_CORE):
            g = 4 * c + j
            b, w = g // NW, g % NW
            out[b, w * WS:(w + 1) * WS, :] = y[j].T
    return out



# revision 11
# speedup vs baseline: 1.4644x; 1.0342x over previous
"""Trainium2 Bass kernel for windowed (Linformer-style) attention.

Reference computation (per input x in {x1, x2}):
  - split n=6400 into 8 windows of 800; per (batch, window, head): full attention
    with dim_head=16, then concat heads, project with Wout, add bias.
  - final output = path(x1) + path(x2).

Sharding: 2 inputs x 4 batches x 8 windows = 64 independent attention "block-inputs".
Core c gets (b,w) blocks g in {4c..4c+3}, both the x1 and x2 paths (8 block-inputs),
so the x1+x2 sum and output projection happen on-core.

v3 design notes (ACT-exp is the bottleneck engine; everything else hides
under it):
  - All attention matmuls in bf16 (1 cycle/row on the PE vs fp32's 4):
    host ships X^T pre-transposed in bf16; Q^T/K^T/V_aug/E tiles are bf16.
    Accumulation stays fp32 in PSUM; softmax denominator / reciprocal /
    normalization / output projection stay fp32.
  - Globally software-pipelined emission: the strict-FIFO PE queue gets
    S(t+1) BEFORE O(t), so while O(t) waits on exp(t), S(t+1) has already
    run and ACT can start exp(t+1) back-to-back.  QKV / V / xt-DMA of the
    next block-input are hoisted into the exp shadow of the current one,
    and the pair normalization + output projection are injected into the
    first steps of the following unit.
  - V augmented with a ones column (ones row in X^T + WVA layout) so the
    softmax denominator falls out of the O^T = V_aug^T @ E accumulation.
  - Per-head S^T / O^T matmuls use tile_position row/col tiling (4x32
    strips) so heads run concurrently in the PE array on hardware.
  - PSUM budget (8 banks): ps_s 2bufs x [128,1024] (4) for S^T;
    ps_ot 1buf x [128,1024] (2) shared by the O^T accumulator and the
    reciprocal broadcast; ps_p 2bufs x [128,512] (2) for projections.
"""

import os
import sys

sys.path.insert(0, "/opt/trn_rl_repo")

import numpy as np
import ml_dtypes
from contextlib import ExitStack

import concourse.bass as bass
import concourse.mybir as mybir
import concourse.tile as tile
from concourse import bacc
from concourse.bass_utils import run_bass_kernel_spmd

F32 = mybir.dt.float32
BF16 = mybir.dt.bfloat16
AF = mybir.ActivationFunctionType

DIM = 48
HEADS = 4
DH = 16
WS = 800        # window (i dimension)
WSP = 896       # j dimension padded to 7*128
INNER = 64
SCALE = DH ** -0.5  # 0.25
B = 4
N = 6400
NW = 8
NCORES = 8
BLOCKS_PER_CORE = 4
NBI = 8  # block-inputs per core: 4 blocks x 2 inputs
NJ = 7   # j-chunks per window: 7x128 (padded)
NSTEP = NJ * HEADS  # 28 (jc, h) steps per block-input
ICHUNKS = [(0, 512), (512, 800)]    # i (token) chunks, <=512 for fp32 matmul
KCHUNKS = [(0, 512), (512, 896)]    # K^T free-dim chunks

# unit order: pair p = block-inputs (p, p+4) -> process adjacently
UNITS = [0, 4, 1, 5, 2, 6, 3, 7]

USE_TILE_POSITION = os.environ.get("KERNEL_NO_TILEPOS", "0") != "1"
# benchmarking: repeat the whole kernel body R times inside the NEFF so the
# marginal wall-clock per rep isolates device time from dispatch overhead
REPS = int(os.environ.get("KERNEL_REPS", "1"))
# DIAG="noexp": replace the ACT exp with nothing (O reads a constant tile)
# to measure the PE/DVE/DMA pipeline floor. Diagnostic only — wrong results.
DIAG = os.environ.get("KERNEL_DIAG", "")


def _tp(row, col):
    return (row, col) if USE_TILE_POSITION else None


class _Body:
    """Emission helper holding pools/constants for one kernel body."""

    def __init__(self, ctx, tc, y_ap, xt_ap, consts):
        self.nc = tc.nc
        self.tc = tc
        self.y_ap = y_ap
        self.xt_ap = xt_ap
        (self.wq_s, self.wk_s, self.wva_s, self.wouta_s,
         self.b2_s) = consts

        self.xt_pool = ctx.enter_context(tc.tile_pool(name="xt", bufs=2))
        self.qkt_pool = ctx.enter_context(tc.tile_pool(name="qkt", bufs=2))
        self.vaug_pool = ctx.enter_context(tc.tile_pool(name="vaug", bufs=3))
        self.e_pool = ctx.enter_context(tc.tile_pool(name="e", bufs=4))
        self.ots_pool = ctx.enter_context(tc.tile_pool(name="ots", bufs=3))
        self.den_pool = ctx.enter_context(tc.tile_pool(name="den", bufs=2))
        self.ont_pool = ctx.enter_context(tc.tile_pool(name="ont", bufs=3))
        self.rs_pool = ctx.enter_context(tc.tile_pool(name="rs", bufs=2))
        self.outb_pool = ctx.enter_context(tc.tile_pool(name="outb", bufs=2))

        self.ps_s = ctx.enter_context(
            tc.tile_pool(name="ps_s", bufs=2, space="PSUM"))
        self.ps_ot = ctx.enter_context(
            tc.tile_pool(name="ps_ot", bufs=1, space="PSUM"))
        self.ps_p = ctx.enter_context(
            tc.tile_pool(name="ps_p", bufs=2, space="PSUM"))

        # per-unit live state
        self.xts = {}     # bi -> xt tile
        self.qkt = {}     # bi -> qkt tile
        self.va = {}      # (bi, jc) -> V_aug tile
        self.sg = {}      # (bi, t) -> S^T psum tile
        self.ot = {}      # bi -> O^T psum accumulator
        self.ots = {}     # bi -> O^T sbuf copy
        self.den = {}     # pair -> denominator tile
        self.rcp = {}     # pair -> reciprocal tile
        self.onts = {}    # pair -> [ont0, ont1]
        self.e_const = None

    # ---- emission pieces ----
    def emit_xt_dma(self, bi):
        nc = self.nc
        xts = self.xt_pool.tile([49, WSP], BF16, tag="xt", name="xts")
        nc.sync.dma_start(xts[:], self.xt_ap[bi, :, :])
        self.xts[bi] = xts

    def emit_qkv(self, bi, which):
        """which: 0 emits Q^T projection, 1 emits K^T projection."""
        nc = self.nc
        if bi not in self.qkt:
            self.qkt[bi] = self.qkt_pool.tile([128, 2, WSP], BF16, tag="qkt", name="qkt")
        qkt = self.qkt[bi]
        xts = self.xts[bi]
        w_s = self.wq_s if which == 0 else self.wk_s
        chunks = ICHUNKS if which == 0 else KCHUNKS
        for n0, n1 in chunks:
            pp = self.ps_p.tile([128, 512], F32, tag="p", name="pp")
            nc.tensor.matmul(pp[:, 0:n1 - n0], w_s[:, :], xts[0:48, n0:n1],
                             start=True, stop=True)
            nc.vector.tensor_copy(qkt[:, which, n0:n1], pp[:, 0:n1 - n0])

    def emit_v(self, bi, jc):
        nc = self.nc
        j0 = jc * 128
        va = self.vaug_pool.tile([128, 128], BF16, tag="va", name="va")
        vp = self.ps_p.tile([128, 512], F32, tag="p", name="pp")
        nc.tensor.matmul(vp[:, 0:128], self.xts[bi][:, j0:j0 + 128],
                         self.wva_s[:, :], start=True, stop=True)
        nc.vector.tensor_copy(va[:, :], vp[:, 0:128])
        self.va[(bi, jc)] = va

    def emit_s(self, bi, t):
        nc = self.nc
        jc, h = t // HEADS, t % HEADS
        j0 = jc * 128
        qkt = self.qkt[bi]
        sg = self.ps_s.tile([128, 1024], F32, tag="sg", name="sg")
        for n0, n1 in ICHUNKS:
            nc.tensor.matmul(
                sg[:, n0:n1],
                qkt[32 * h:32 * h + 16, 1, j0:j0 + 128],
                qkt[32 * h:32 * h + 16, 0, n0:n1],
                start=True, stop=True,
                tile_position=_tp(32 * h, 0),
            )
        self.sg[(bi, t)] = sg

    def emit_exp(self, bi, t):
        nc = self.nc
        sg = self.sg.pop((bi, t))
        if DIAG == "noexp":
            if self.e_const is None:
                self.e_const = self.e_pool.tile([128, WS], BF16, tag="ec",
                                                name="ec")
                nc.vector.memset(self.e_const[:], 1.0)
            return self.e_const
        e = self.e_pool.tile([128, WS], BF16, tag="e", name="e")
        nc.scalar.activation(e[:, :], sg[:, 0:WS], AF.Exp, scale=float(SCALE))
        return e

    def emit_o(self, bi, t, e):
        nc = self.nc
        jc, h = t // HEADS, t % HEADS
        if bi not in self.ot:
            self.ot[bi] = self.ps_ot.tile([128, 1024], F32, tag="ot", name="ot")
        ot = self.ot[bi]
        va = self.va[(bi, jc)]
        for n0, n1 in ICHUNKS:
            nc.tensor.matmul(
                ot[32 * h:32 * h + 32, n0:n1],
                va[:, 32 * h:32 * h + 32],
                e[:, n0:n1],
                start=(jc == 0), stop=(jc == NJ - 1),
                tile_position=_tp(0, 32 * h),
            )
        if h == HEADS - 1:
            del self.va[(bi, jc)]

    def emit_epilogue(self, bi, half, pair):
        """PSUM -> SBUF copy of O^T, denominator row gather."""
        nc = self.nc
        ot = self.ot.pop(bi)
        self.qkt.pop(bi, None)
        self.xts.pop(bi, None)
        ots = self.ots_pool.tile([128, WS], F32, tag="ots", name="ots")
        nc.vector.tensor_copy(ots[:], ot[:, 0:WS])
        self.ots[bi] = ots
        if (pair, half) not in self.den:
            self.den[(pair, half)] = self.den_pool.tile([4, WS], F32,
                                                        tag="den", name="den")
        den = self.den[(pair, half)]
        for h in range(HEADS):
            nc.sync.dma_start(den[h:h + 1, :],
                              ots[32 * h + 16:32 * h + 17, :])

    def emit_recip_half(self, pair, half):
        """Reciprocal of one half's denominator rows (DVE), then broadcast
        them to the O^T partition layout with 8 parallel stride-0 DMAs.
        Running this at the owning unit's end hides the ~17us broadcast
        latency (128 x 1600B descriptors) under the next unit's compute."""
        nc = self.nc
        den = self.den.pop((pair, half))
        rcp = self.den_pool.tile([4, WS], F32, tag="rcp", name="rcp")
        scr = self.den_pool.tile([4, WS], F32, tag="scr", name="scr")
        nc.vector.reciprocal_approx_accurate(rcp[:], den[0:4, :], scr[:])
        rcpb = self.den_pool.tile([4, WS], BF16, tag="rcpb", name="rcpb")
        nc.vector.tensor_copy(rcpb[:], rcp[:])
        rs = self.rs_pool.tile([128, WS], BF16, tag="rs", name="rs")
        for k in range(8):
            r = k // 2
            src = rcpb[r:r + 1, :].unsqueeze(1)
            nc.sync.dma_start(rs[16 * k:16 * k + 16, :],
                              src.to_broadcast([1, 16, WS]))
        self.rcp[(pair, half)] = rs

    def emit_norm(self, pair):
        """Normalize on the DVE (broadcasts issued at each unit's end),
        then pre-sum the two halves so the projection runs half the
        matmuls."""
        nc = self.nc
        onts = []
        for half in range(2):
            bi = pair + 4 * half
            rs = self.rcp.pop((pair, half))
            ont = self.ont_pool.tile([128, WS], BF16, tag="ont", name="ont")
            nc.vector.tensor_mul(ont[:], self.ots.pop(bi)[:], rs[:])
            onts.append(ont)
        osum = self.ont_pool.tile([128, WS], BF16, tag="osum", name="osum")
        nc.vector.tensor_add(osum[:], onts[0][:], onts[1][:])
        self.onts[pair] = osum

    def emit_outproj(self, pair):
        """out^T = Wout_all^T @ (ont0 + ont1): 4 matmuls sharing one LDW,
        bias via per-partition scalar add, transposed DMA to DRAM."""
        nc = self.nc
        osum = self.onts.pop(pair)
        outb = self.outb_pool.tile([48, WS], F32, tag="outb", name="outb")
        for n0, n1 in ICHUNKS:
            op = self.ps_p.tile([128, 512], F32, tag="p", name="pp")
            nc.tensor.matmul(op[0:48, 0:n1 - n0], self.wouta_s[:, :],
                             osum[:, n0:n1], start=True, stop=True)
            nc.vector.tensor_scalar_add(outb[:, n0:n1], op[0:48, 0:n1 - n0],
                                        self.b2_s[:, 0:1])
        nc.sync.dma_start(self.y_ap[pair], outb[:])


def build_kernel_body(ctx, tc, y_ap, xt_ap, wq_ap, wk_ap, wva_ap, wouta_ap,
                      b2_ap):
    nc = tc.nc
    consts = ctx.enter_context(tc.tile_pool(name="consts", bufs=1))
    wq_s = consts.tile([48, 128], BF16, tag="wq")
    nc.sync.dma_start(wq_s[:], wq_ap[:, :])
    wk_s = consts.tile([48, 128], BF16, tag="wk")
    nc.sync.dma_start(wk_s[:], wk_ap[:, :])
    wva_s = consts.tile([49, 128], BF16, tag="wva")
    nc.sync.dma_start(wva_s[:], wva_ap[:, :])
    wouta_s = consts.tile([128, 48], BF16, tag="wouta")
    nc.sync.dma_start(wouta_s[:], wouta_ap[:, :])
    b2_s = consts.tile([48, 1], F32, tag="b2")
    nc.sync.dma_start(b2_s[:], b2_ap[:, :])

    body = _Body(ctx, tc, y_ap, xt_ap,
                 (wq_s, wk_s, wva_s, wouta_s, b2_s))

    for _rep in range(REPS):
        _emit_pipeline(body)


def _emit_pipeline(body):
    """Flattened, software-pipelined emission over all 8 block-input units."""
    # prologue for the first unit
    bi0 = UNITS[0]
    body.emit_xt_dma(bi0)
    body.emit_qkv(bi0, 0)
    body.emit_qkv(bi0, 1)
    body.emit_v(bi0, 0)
    body.emit_s(bi0, 0)

    for u, bi in enumerate(UNITS):
        nxt = UNITS[u + 1] if u + 1 < len(UNITS) else None
        for t in range(NSTEP):
            # hoisted prologue work for the next unit, placed in the exp
            # shadow near the end of this unit
            if nxt is not None:
                if t == 20:
                    body.emit_xt_dma(nxt)
                elif t == 22:
                    body.emit_qkv(nxt, 0)
                elif t == 24:
                    body.emit_qkv(nxt, 1)
            # deferred normalization of the previous pair, injected early in
            # this unit so its PE/DVE work hides under this unit's exp stream
            if u % 2 == 0 and u > 0:
                if t == 2:
                    body.emit_norm(u // 2 - 1)
                elif t == 8:
                    body.emit_outproj(u // 2 - 1)
            # emit S one step ahead of O so the in-order PE queue never has
            # an un-issued S behind a stalled O
            if t + 1 < NSTEP:
                if (t + 1) % HEADS == 0:
                    body.emit_v(bi, (t + 1) // HEADS)
                body.emit_s(bi, t + 1)
            elif nxt is not None:
                body.emit_v(nxt, 0)
                body.emit_s(nxt, 0)
            e = body.emit_exp(bi, t)
            body.emit_o(bi, t, e)
        body.emit_epilogue(bi, half=u % 2, pair=u // 2)
        body.emit_recip_half(u // 2, u % 2)

    # final pair's normalization runs in the tail
    last_pair = len(UNITS) // 2 - 1
    body.emit_norm(last_pair)
    body.emit_outproj(last_pair)


def _dedup_ldweights(nc):
    """Drop InstLdweights that reload the exact weights the PE already holds.

    tile_legalize splits every non-f32 matmul into LDW+MM; chunked matmuls
    sharing one stationary operand then carry a redundant second LDW (no
    waits/updates).  Removing it saves ~110ns of PE sequencer time each."""
    fn = nc.m.functions[0]
    for bb in fn.blocks:
        insts = bb.instructions
        drop = []
        prev_key = None
        for k in range(len(insts)):
            inst = insts[k]
            tn = type(inst).__name__
            if str(inst.engine) != "EngineType.PE":
                continue
            if tn == "InstLdweights":
                w = inst.ins[0]
                key = (w.memsetref, w.offset, str(w.ap), str(w.dtype),
                       str(inst.tile_position), str(inst.tile_size),
                       str(inst.perf_mode))
                si = inst.sync_info
                clean = si is None or (not si.on_wait and not si.on_update)
                if key == prev_key and clean:
                    drop.append(k)
                    continue
                prev_key = key
            elif tn == "InstMatmult":
                pass  # same weights keep streaming; array not clobbered
            else:
                prev_key = None
        for k in reversed(drop):
            del insts[k]


_CACHED = {}


def build_nc():
    key = (REPS, USE_TILE_POSITION, DIAG)
    if key in _CACHED:
        return _CACHED[key]
    nc = bacc.Bacc("TRN2", target_bir_lowering=False, debug=False)
    xt = nc.dram_tensor("xt", [NBI, 49, WSP], BF16, kind="ExternalInput")
    wq = nc.dram_tensor("wq", [48, 128], BF16, kind="ExternalInput")
    wk = nc.dram_tensor("wk", [48, 128], BF16, kind="ExternalInput")
    wva = nc.dram_tensor("wva", [49, 128], BF16, kind="ExternalInput")
    wouta = nc.dram_tensor("wouta", [128, 48], BF16, kind="ExternalInput")
    b2 = nc.dram_tensor("b2", [48, 1], F32, kind="ExternalInput")
    y = nc.dram_tensor("y", [BLOCKS_PER_CORE, DIM, WS], F32,
                       kind="ExternalOutput")
    with tile.TileContext(nc) as tc:
        with ExitStack() as ctx:
            build_kernel_body(ctx, tc, y.ap(), xt.ap(), wq.ap(), wk.ap(),
                              wva.ap(), wouta.ap(), b2.ap())
    nc.compile()
    _dedup_ldweights(nc)
    _CACHED[key] = nc
    return nc


def _prep_consts(Wqkv, Wout, bout):
    WQ = np.zeros((48, 128), np.float32)
    WK = np.zeros((48, 128), np.float32)
    WVA = np.zeros((49, 128), np.float32)
    WOUTA = np.zeros((128, 48), np.float32)
    for h in range(HEADS):
        WQ[:, 32 * h:32 * h + 16] = Wqkv[h * 48:h * 48 + 16, :].T
        WK[:, 32 * h:32 * h + 16] = Wqkv[h * 48 + 16:h * 48 + 32, :].T
        WVA[0:48, 32 * h:32 * h + 16] = Wqkv[h * 48 + 32:h * 48 + 48, :].T
        WVA[48, 32 * h + 16] = 1.0
        WOUTA[32 * h:32 * h + 16, :] = Wout[:, 16 * h:16 * h + 16].T
    B2 = (2.0 * bout).astype(np.float32)[:, None]
    BF = ml_dtypes.bfloat16
    return (WQ.astype(BF), WK.astype(BF), WVA.astype(BF), WOUTA.astype(BF),
            B2)


def kernel(x1, x2, Wqkv, Wout, bout):
    x1 = np.ascontiguousarray(x1, np.float32)
    x2 = np.ascontiguousarray(x2, np.float32)
    Wqkv = np.asarray(Wqkv, np.float32)
    Wout = np.asarray(Wout, np.float32)
    bout = np.asarray(bout, np.float32)

    nc = build_nc()
    WQ, WK, WVA, WOUTA, B2 = _prep_consts(Wqkv, Wout, bout)

    in_maps = []
    for c in range(NCORES):
        XT = np.zeros((NBI, 49, WSP), ml_dtypes.bfloat16)
        for j in range(BLOCKS_PER_CORE):
            g = 4 * c + j
            b, w = g // NW, g % NW
            XT[j, 0:48, 0:WS] = x1[b, w * WS:(w + 1) * WS, :].T
            XT[j, 48, 0:WS] = 1.0
            XT[4 + j, 0:48, 0:WS] = x2[b, w * WS:(w + 1) * WS, :].T
            XT[4 + j, 48, 0:WS] = 1.0
        in_maps.append({
            "xt": XT, "wq": WQ, "wk": WK, "wva": WVA,
            "wouta": WOUTA, "b2": B2,
        })

    res = run_bass_kernel_spmd(nc, in_maps, core_ids=list(range(NCORES)))
    kernel._last_results = res

    out = np.empty((B, N, DIM), np.float32)
    for c in range(NCORES):
        y = res.results[c]["y"]
        for j in range(BLOCKS_PER_CORE):
            g = 4 * c + j
            b, w = g // NW, g % NW
            out[b, w * WS:(w + 1) * WS, :] = y[j].T
    return out



# revision 13
# speedup vs baseline: 1.5060x; 1.0284x over previous
"""Trainium2 Bass kernel for windowed (Linformer-style) attention.

Reference computation (per input x in {x1, x2}):
  - split n=6400 into 8 windows of 800; per (batch, window, head): full attention
    with dim_head=16, then concat heads, project with Wout, add bias.
  - final output = path(x1) + path(x2).

Sharding: 2 inputs x 4 batches x 8 windows = 64 independent attention "block-inputs".
Core c gets (b,w) blocks g in {4c..4c+3}, both the x1 and x2 paths (8 block-inputs),
so the x1+x2 sum and output projection happen on-core.

v3 design notes (ACT-exp is the bottleneck engine; everything else hides
under it):
  - All attention matmuls in bf16 (1 cycle/row on the PE vs fp32's 4):
    host ships X^T pre-transposed in bf16; Q^T/K^T/V_aug/E tiles are bf16.
    Accumulation stays fp32 in PSUM; softmax denominator / reciprocal /
    normalization / output projection stay fp32.
  - Globally software-pipelined emission: the strict-FIFO PE queue gets
    S(t+1) BEFORE O(t), so while O(t) waits on exp(t), S(t+1) has already
    run and ACT can start exp(t+1) back-to-back.  QKV / V / xt-DMA of the
    next block-input are hoisted into the exp shadow of the current one,
    and the pair normalization + output projection are injected into the
    first steps of the following unit.
  - V augmented with a ones column (ones row in X^T + WVA layout) so the
    softmax denominator falls out of the O^T = V_aug^T @ E accumulation.
  - Per-head S^T / O^T matmuls use tile_position row/col tiling (4x32
    strips) so heads run concurrently in the PE array on hardware.
  - PSUM budget (8 banks): ps_s 2bufs x [128,1024] (4) for S^T;
    ps_ot 1buf x [128,1024] (2) shared by the O^T accumulator and the
    reciprocal broadcast; ps_p 2bufs x [128,512] (2) for projections.
"""

import os
import sys

sys.path.insert(0, "/opt/trn_rl_repo")

import numpy as np
import ml_dtypes
from contextlib import ExitStack

import concourse.bass as bass
import concourse.mybir as mybir
import concourse.tile as tile
from concourse import bacc
from concourse.bass_utils import run_bass_kernel_spmd

F32 = mybir.dt.float32
BF16 = mybir.dt.bfloat16
AF = mybir.ActivationFunctionType

DIM = 48
HEADS = 4
DH = 16
WS = 800        # window (i dimension)
WSP = 896       # j dimension padded to 7*128
INNER = 64
SCALE = DH ** -0.5  # 0.25
B = 4
N = 6400
NW = 8
NCORES = 8
BLOCKS_PER_CORE = 4
NBI = 8  # block-inputs per core: 4 blocks x 2 inputs
NJ = 7   # j-chunks per window: 7x128 (padded)
NSTEP = NJ * HEADS  # 28 (jc, h) steps per block-input
ICHUNKS = [(0, 512), (512, 800)]    # i (token) chunks, <=512 for fp32 matmul
KCHUNKS = [(0, 512), (512, 896)]    # K^T free-dim chunks

# unit order: pair p = block-inputs (p, p+4) -> process adjacently
UNITS = [0, 4, 1, 5, 2, 6, 3, 7]

USE_TILE_POSITION = os.environ.get("KERNEL_NO_TILEPOS", "0") != "1"
# benchmarking: repeat the whole kernel body R times inside the NEFF so the
# marginal wall-clock per rep isolates device time from dispatch overhead
REPS = int(os.environ.get("KERNEL_REPS", "1"))
# DIAG="noexp": replace the ACT exp with nothing (O reads a constant tile)
# to measure the PE/DVE/DMA pipeline floor. Diagnostic only — wrong results.
DIAG = os.environ.get("KERNEL_DIAG", "")


def _tp(row, col):
    return (row, col) if USE_TILE_POSITION else None


class _Body:
    """Emission helper holding pools/constants for one kernel body."""

    def __init__(self, ctx, tc, y_ap, xt_ap, consts):
        self.nc = tc.nc
        self.tc = tc
        self.y_ap = y_ap
        self.xt_ap = xt_ap
        (self.wq_s, self.wk_s, self.wva_s, self.wouta_s,
         self.b2_s) = consts

        self.xt_pool = ctx.enter_context(tc.tile_pool(name="xt", bufs=2))
        self.qkt_pool = ctx.enter_context(tc.tile_pool(name="qkt", bufs=2))
        self.vaug_pool = ctx.enter_context(tc.tile_pool(name="vaug", bufs=9))
        self.e_pool = ctx.enter_context(tc.tile_pool(name="e", bufs=4))
        self.ots_pool = ctx.enter_context(tc.tile_pool(name="ots", bufs=3))
        self.den_pool = ctx.enter_context(tc.tile_pool(name="den", bufs=2))
        self.ont_pool = ctx.enter_context(tc.tile_pool(name="ont", bufs=3))
        self.rs_pool = ctx.enter_context(tc.tile_pool(name="rs", bufs=2))
        self.outb_pool = ctx.enter_context(tc.tile_pool(name="outb", bufs=2))

        self.ps_s = ctx.enter_context(
            tc.tile_pool(name="ps_s", bufs=2, space="PSUM"))
        self.ps_ot = ctx.enter_context(
            tc.tile_pool(name="ps_ot", bufs=1, space="PSUM"))
        self.ps_p = ctx.enter_context(
            tc.tile_pool(name="ps_p", bufs=2, space="PSUM"))

        # per-unit live state
        self.xts = {}     # bi -> xt tile
        self.qkt = {}     # bi -> qkt tile
        self.va = {}      # (bi, jc) -> V_aug tile
        self.sg = {}      # (bi, t) -> S^T psum tile
        self.ot = {}      # bi -> O^T psum accumulator
        self.ots = {}     # bi -> O^T sbuf copy
        self.den = {}     # pair -> denominator tile
        self.rcp = {}     # pair -> reciprocal tile
        self.onts = {}    # pair -> [ont0, ont1]
        self.e_const = None

    # ---- emission pieces ----
    def emit_xt_dma(self, bi):
        nc = self.nc
        xts = self.xt_pool.tile([49, WSP], BF16, tag="xt", name="xts")
        nc.sync.dma_start(xts[:], self.xt_ap[bi, :, :])
        self.xts[bi] = xts

    def emit_qkv(self, bi, which):
        """which: 0 emits Q^T projection, 1 emits K^T projection."""
        nc = self.nc
        if bi not in self.qkt:
            self.qkt[bi] = self.qkt_pool.tile([128, 2, WSP], BF16, tag="qkt", name="qkt")
        qkt = self.qkt[bi]
        xts = self.xts[bi]
        w_s = self.wq_s if which == 0 else self.wk_s
        chunks = ICHUNKS if which == 0 else KCHUNKS
        for n0, n1 in chunks:
            pp = self.ps_p.tile([128, 512], F32, tag="p", name="pp")
            nc.tensor.matmul(pp[:, 0:n1 - n0], w_s[:, :], xts[0:48, n0:n1],
                             start=True, stop=True)
            nc.vector.tensor_copy(qkt[:, which, n0:n1], pp[:, 0:n1 - n0])

    def emit_v(self, bi, jc):
        nc = self.nc
        j0 = jc * 128
        va = self.vaug_pool.tile([128, 128], BF16, tag="va", name="va")
        vp = self.ps_p.tile([128, 512], F32, tag="p", name="pp")
        nc.tensor.matmul(vp[:, 0:128], self.xts[bi][:, j0:j0 + 128],
                         self.wva_s[:, :], start=True, stop=True)
        nc.vector.tensor_copy(va[:, :], vp[:, 0:128])
        self.va[(bi, jc)] = va

    def emit_s(self, bi, t):
        nc = self.nc
        h, jc = t // NJ, t % NJ
        j0 = jc * 128
        qkt = self.qkt[bi]
        sg = self.ps_s.tile([128, 1024], F32, tag="sg", name="sg")
        for n0, n1 in ICHUNKS:
            nc.tensor.matmul(
                sg[:, n0:n1],
                qkt[32 * h:32 * h + 16, 1, j0:j0 + 128],
                qkt[32 * h:32 * h + 16, 0, n0:n1],
                start=True, stop=True,
                tile_position=_tp(32 * h, 0),
            )
        self.sg[(bi, t)] = sg

    def emit_exp(self, bi, t):
        nc = self.nc
        sg = self.sg.pop((bi, t))
        if DIAG == "noexp":
            if self.e_const is None:
                self.e_const = self.e_pool.tile([128, WS], BF16, tag="ec",
                                                name="ec")
                nc.vector.memset(self.e_const[:], 1.0)
            return self.e_const
        e = self.e_pool.tile([128, WS], BF16, tag="e", name="e")
        nc.scalar.activation(e[:, :], sg[:, 0:WS], AF.Exp, scale=float(SCALE))
        return e

    def emit_o(self, bi, t, e):
        nc = self.nc
        h, jc = t // NJ, t % NJ
        if bi not in self.ot:
            self.ot[bi] = self.ps_ot.tile([128, 1024], F32, tag="ot", name="ot")
        ot = self.ot[bi]
        va = self.va[(bi, jc)]
        for n0, n1 in ICHUNKS:
            nc.tensor.matmul(
                ot[32 * h:32 * h + 32, n0:n1],
                va[:, 32 * h:32 * h + 32],
                e[:, n0:n1],
                start=(jc == 0), stop=(jc == NJ - 1),
                tile_position=_tp(0, 32 * h),
            )
        if h == HEADS - 1:
            del self.va[(bi, jc)]
        return h, jc

    def emit_head_epilogue(self, bi, h, half, pair):
        """Head h's O^T strip is complete: copy it PSUM -> SBUF and gather
        its denominator row.  Spreading this through the unit (h-outer step
        order) leaves only head 3's chain on the kernel tail."""
        nc = self.nc
        ot = self.ot[bi] if h < HEADS - 1 else self.ot.pop(bi)
        if bi not in self.ots:
            self.ots[bi] = self.ots_pool.tile([128, WS], F32, tag="ots",
                                              name="ots")
        ots = self.ots[bi]
        nc.vector.tensor_copy(ots[32 * h:32 * h + 32, :],
                              ot[32 * h:32 * h + 32, 0:WS])
        if (pair, half) not in self.den:
            self.den[(pair, half)] = self.den_pool.tile([4, WS], F32,
                                                        tag="den", name="den")
        den = self.den[(pair, half)]
        nc.sync.dma_start(den[h:h + 1, :], ots[32 * h + 16:32 * h + 17, :])

    def emit_epilogue(self, bi):
        self.qkt.pop(bi, None)
        self.xts.pop(bi, None)

    def emit_recip_half(self, pair, half):
        """Reciprocal of one half's denominator rows (DVE), then broadcast
        them to the O^T partition layout with 8 parallel stride-0 DMAs.
        Running this at the owning unit's end hides the ~17us broadcast
        latency (128 x 1600B descriptors) under the next unit's compute."""
        nc = self.nc
        den = self.den.pop((pair, half))
        rcp = self.den_pool.tile([4, WS], F32, tag="rcp", name="rcp")
        scr = self.den_pool.tile([4, WS], F32, tag="scr", name="scr")
        nc.vector.reciprocal_approx_accurate(rcp[:], den[0:4, :], scr[:])
        rcpb = self.den_pool.tile([4, WS], BF16, tag="rcpb", name="rcpb")
        nc.vector.tensor_copy(rcpb[:], rcp[:])
        if (pair, half) == (3, 1):
            # kernel tail: broadcast via a tiny PE matmul (block-indicator
            # stationary) into PSUM - the DMA broadcast's issue+transfer
            # latency (~9us) would sit exposed on the critical path
            ps_rs = self.ps_s.tile([128, 1024], F32, tag="sg", name="ps_rs")
            for n0, n1 in ICHUNKS:
                nc.tensor.matmul(ps_rs[:, n0:n1], self.bsel_s[:, :],
                                 rcpb[:, n0:n1], start=True, stop=True)
            self.rcp[(pair, half)] = ps_rs
        else:
            rs = self.rs_pool.tile([128, WS], BF16, tag="rs", name="rs")
            for k in range(8):
                r = k // 2
                src = rcpb[r:r + 1, :].unsqueeze(1)
                nc.sync.dma_start(rs[16 * k:16 * k + 16, :],
                                  src.to_broadcast([1, 16, WS]))
            self.rcp[(pair, half)] = rs

    def emit_norm(self, pair):
        """Normalize on the DVE (broadcasts issued at each unit's end),
        then pre-sum the two halves so the projection runs half the
        matmuls."""
        nc = self.nc
        onts = []
        for half in range(2):
            bi = pair + 4 * half
            rs = self.rcp.pop((pair, half))
            ont = self.ont_pool.tile([128, WS], BF16, tag="ont", name="ont")
            if (pair, half) == (3, 1):
                nc.vector.tensor_mul(ont[:], self.ots.pop(bi)[:],
                                     rs[:, 0:WS])
            else:
                nc.vector.tensor_mul(ont[:], self.ots.pop(bi)[:], rs[:])
            onts.append(ont)
        osum = self.ont_pool.tile([128, WS], BF16, tag="osum", name="osum")
        nc.vector.tensor_add(osum[:], onts[0][:], onts[1][:])
        self.onts[pair] = osum

    def emit_outproj(self, pair):
        """out^T = Wout_all^T @ (ont0 + ont1): 4 matmuls sharing one LDW,
        bias via per-partition scalar add, transposed DMA to DRAM."""
        nc = self.nc
        osum = self.onts.pop(pair)
        outb = self.outb_pool.tile([48, WS], F32, tag="outb", name="outb")
        for n0, n1 in ICHUNKS:
            op = self.ps_p.tile([128, 512], F32, tag="p", name="pp")
            nc.tensor.matmul(op[0:48, 0:n1 - n0], self.wouta_s[:, :],
                             osum[:, n0:n1], start=True, stop=True)
            nc.vector.tensor_scalar_add(outb[:, n0:n1], op[0:48, 0:n1 - n0],
                                        self.b2_s[:, 0:1])
        nc.sync.dma_start(self.y_ap[pair], outb[:])


def build_kernel_body(ctx, tc, y_ap, xt_ap, wq_ap, wk_ap, wva_ap, wouta_ap,
                      b2_ap, bsel_ap):
    nc = tc.nc
    consts = ctx.enter_context(tc.tile_pool(name="consts", bufs=1))
    bsel_s = consts.tile([4, 128], BF16, tag="bsel")
    nc.sync.dma_start(bsel_s[:], bsel_ap[:, :])
    wq_s = consts.tile([48, 128], BF16, tag="wq")
    nc.sync.dma_start(wq_s[:], wq_ap[:, :])
    wk_s = consts.tile([48, 128], BF16, tag="wk")
    nc.sync.dma_start(wk_s[:], wk_ap[:, :])
    wva_s = consts.tile([49, 128], BF16, tag="wva")
    nc.sync.dma_start(wva_s[:], wva_ap[:, :])
    wouta_s = consts.tile([128, 48], BF16, tag="wouta")
    nc.sync.dma_start(wouta_s[:], wouta_ap[:, :])
    b2_s = consts.tile([48, 1], F32, tag="b2")
    nc.sync.dma_start(b2_s[:], b2_ap[:, :])

    body = _Body(ctx, tc, y_ap, xt_ap,
                 (wq_s, wk_s, wva_s, wouta_s, b2_s))
    body.bsel_s = bsel_s

    for _rep in range(REPS):
        _emit_pipeline(body)


def _emit_pipeline(body):
    """Flattened, software-pipelined emission over all 8 block-input units."""
    # prologue for the first unit
    bi0 = UNITS[0]
    body.emit_xt_dma(bi0)
    body.emit_qkv(bi0, 0)
    body.emit_qkv(bi0, 1)
    body.emit_v(bi0, 0)
    body.emit_s(bi0, 0)

    for u, bi in enumerate(UNITS):
        nxt = UNITS[u + 1] if u + 1 < len(UNITS) else None
        for t in range(NSTEP):
            # hoisted prologue work for the next unit, placed in the exp
            # shadow near the end of this unit
            if nxt is not None:
                if t == 20:
                    body.emit_xt_dma(nxt)
                elif t == 22:
                    body.emit_qkv(nxt, 0)
                elif t == 24:
                    body.emit_qkv(nxt, 1)
            # deferred normalization of the previous pair, injected early in
            # this unit so its PE/DVE work hides under this unit's exp stream
            if u % 2 == 0 and u > 0:
                if t == 2:
                    body.emit_norm(u // 2 - 1)
                elif t == 8:
                    body.emit_outproj(u // 2 - 1)
            # emit S one step ahead of O so the in-order PE queue never has
            # an un-issued S behind a stalled O
            if 1 <= t + 1 <= 6:
                body.emit_v(bi, t + 1)
            if t + 1 < NSTEP:
                body.emit_s(bi, t + 1)
            elif nxt is not None:
                body.emit_v(nxt, 0)
                body.emit_s(nxt, 0)
            e = body.emit_exp(bi, t)
            h, jc = body.emit_o(bi, t, e)
            if jc == NJ - 1:
                body.emit_head_epilogue(bi, h, half=u % 2, pair=u // 2)
                if h == HEADS - 1:
                    body.emit_recip_half(u // 2, u % 2)
        body.emit_epilogue(bi)

    # final pair's normalization runs in the tail
    last_pair = len(UNITS) // 2 - 1
    body.emit_norm(last_pair)
    body.emit_outproj(last_pair)


def _dedup_ldweights(nc):
    """Drop InstLdweights that reload the exact weights the PE already holds.

    tile_legalize splits every non-f32 matmul into LDW+MM; chunked matmuls
    sharing one stationary operand then carry a redundant second LDW (no
    waits/updates).  Removing it saves ~110ns of PE sequencer time each."""
    fn = nc.m.functions[0]
    for bb in fn.blocks:
        insts = bb.instructions
        drop = []
        prev_key = None
        for k in range(len(insts)):
            inst = insts[k]
            tn = type(inst).__name__
            if str(inst.engine) != "EngineType.PE":
                continue
            if tn == "InstLdweights":
                w = inst.ins[0]
                key = (w.memsetref, w.offset, str(w.ap), str(w.dtype),
                       str(inst.tile_position), str(inst.tile_size),
                       str(inst.perf_mode))
                si = inst.sync_info
                clean = si is None or (not si.on_wait and not si.on_update)
                if key == prev_key and clean:
                    drop.append(k)
                    continue
                prev_key = key
            elif tn == "InstMatmult":
                pass  # same weights keep streaming; array not clobbered
            else:
                prev_key = None
        for k in reversed(drop):
            del insts[k]


_CACHED = {}


def build_nc():
    key = (REPS, USE_TILE_POSITION, DIAG)
    if key in _CACHED:
        return _CACHED[key]
    nc = bacc.Bacc("TRN2", target_bir_lowering=False, debug=False)
    xt = nc.dram_tensor("xt", [NBI, 49, WSP], BF16, kind="ExternalInput")
    wq = nc.dram_tensor("wq", [48, 128], BF16, kind="ExternalInput")
    wk = nc.dram_tensor("wk", [48, 128], BF16, kind="ExternalInput")
    wva = nc.dram_tensor("wva", [49, 128], BF16, kind="ExternalInput")
    wouta = nc.dram_tensor("wouta", [128, 48], BF16, kind="ExternalInput")
    b2 = nc.dram_tensor("b2", [48, 1], F32, kind="ExternalInput")
    bsel = nc.dram_tensor("bsel", [4, 128], BF16, kind="ExternalInput")
    y = nc.dram_tensor("y", [BLOCKS_PER_CORE, DIM, WS], F32,
                       kind="ExternalOutput")
    with tile.TileContext(nc) as tc:
        with ExitStack() as ctx:
            build_kernel_body(ctx, tc, y.ap(), xt.ap(), wq.ap(), wk.ap(),
                              wva.ap(), wouta.ap(), b2.ap(), bsel.ap())
    nc.compile()
    _dedup_ldweights(nc)
    _CACHED[key] = nc
    return nc


def _prep_consts(Wqkv, Wout, bout):
    WQ = np.zeros((48, 128), np.float32)
    WK = np.zeros((48, 128), np.float32)
    WVA = np.zeros((49, 128), np.float32)
    WOUTA = np.zeros((128, 48), np.float32)
    for h in range(HEADS):
        WQ[:, 32 * h:32 * h + 16] = Wqkv[h * 48:h * 48 + 16, :].T
        WK[:, 32 * h:32 * h + 16] = Wqkv[h * 48 + 16:h * 48 + 32, :].T
        WVA[0:48, 32 * h:32 * h + 16] = Wqkv[h * 48 + 32:h * 48 + 48, :].T
        WVA[48, 32 * h + 16] = 1.0
        WOUTA[32 * h:32 * h + 16, :] = Wout[:, 16 * h:16 * h + 16].T
    B2 = (2.0 * bout).astype(np.float32)[:, None]
    BSEL = np.zeros((4, 128), np.float32)
    for r in range(4):
        BSEL[r, 32 * r:32 * r + 32] = 1.0
    BF = ml_dtypes.bfloat16
    return (WQ.astype(BF), WK.astype(BF), WVA.astype(BF), WOUTA.astype(BF),
            B2, BSEL.astype(BF))


def kernel(x1, x2, Wqkv, Wout, bout):
    x1 = np.ascontiguousarray(x1, np.float32)
    x2 = np.ascontiguousarray(x2, np.float32)
    Wqkv = np.asarray(Wqkv, np.float32)
    Wout = np.asarray(Wout, np.float32)
    bout = np.asarray(bout, np.float32)

    nc = build_nc()
    WQ, WK, WVA, WOUTA, B2, BSEL = _prep_consts(Wqkv, Wout, bout)

    in_maps = []
    for c in range(NCORES):
        XT = np.zeros((NBI, 49, WSP), ml_dtypes.bfloat16)
        for j in range(BLOCKS_PER_CORE):
            g = 4 * c + j
            b, w = g // NW, g % NW
            XT[j, 0:48, 0:WS] = x1[b, w * WS:(w + 1) * WS, :].T
            XT[j, 48, 0:WS] = 1.0
            XT[4 + j, 0:48, 0:WS] = x2[b, w * WS:(w + 1) * WS, :].T
            XT[4 + j, 48, 0:WS] = 1.0
        in_maps.append({
            "xt": XT, "wq": WQ, "wk": WK, "wva": WVA,
            "wouta": WOUTA, "b2": B2, "bsel": BSEL,
        })

    res = run_bass_kernel_spmd(nc, in_maps, core_ids=list(range(NCORES)))
    kernel._last_results = res

    out = np.empty((B, N, DIM), np.float32)
    for c in range(NCORES):
        y = res.results[c]["y"]
        for j in range(BLOCKS_PER_CORE):
            g = 4 * c + j
            b, w = g // NW, g % NW
            out[b, w * WS:(w + 1) * WS, :] = y[j].T
    return out



# revision 14
# speedup vs baseline: 1.5184x; 1.0083x over previous
"""Trainium2 Bass kernel for windowed (Linformer-style) attention.

Reference computation (per input x in {x1, x2}):
  - split n=6400 into 8 windows of 800; per (batch, window, head): full attention
    with dim_head=16, then concat heads, project with Wout, add bias.
  - final output = path(x1) + path(x2).

Sharding: 2 inputs x 4 batches x 8 windows = 64 independent attention "block-inputs".
Core c gets (b,w) blocks g in {4c..4c+3}, both the x1 and x2 paths (8 block-inputs),
so the x1+x2 sum and output projection happen on-core.

v3 design notes (ACT-exp is the bottleneck engine; everything else hides
under it):
  - All attention matmuls in bf16 (1 cycle/row on the PE vs fp32's 4):
    host ships X^T pre-transposed in bf16; Q^T/K^T/V_aug/E tiles are bf16.
    Accumulation stays fp32 in PSUM; softmax denominator / reciprocal /
    normalization / output projection stay fp32.
  - Globally software-pipelined emission: the strict-FIFO PE queue gets
    S(t+1) BEFORE O(t), so while O(t) waits on exp(t), S(t+1) has already
    run and ACT can start exp(t+1) back-to-back.  QKV / V / xt-DMA of the
    next block-input are hoisted into the exp shadow of the current one,
    and the pair normalization + output projection are injected into the
    first steps of the following unit.
  - V augmented with a ones column (ones row in X^T + WVA layout) so the
    softmax denominator falls out of the O^T = V_aug^T @ E accumulation.
  - Per-head S^T / O^T matmuls use tile_position row/col tiling (4x32
    strips) so heads run concurrently in the PE array on hardware.
  - PSUM budget (8 banks): ps_s 2bufs x [128,1024] (4) for S^T;
    ps_ot 1buf x [128,1024] (2) shared by the O^T accumulator and the
    reciprocal broadcast; ps_p 2bufs x [128,512] (2) for projections.
"""

import os
import sys

sys.path.insert(0, "/opt/trn_rl_repo")

import numpy as np
import ml_dtypes
from contextlib import ExitStack

import concourse.bass as bass
import concourse.mybir as mybir
import concourse.tile as tile
from concourse import bacc
from concourse.bass_utils import run_bass_kernel_spmd

F32 = mybir.dt.float32
BF16 = mybir.dt.bfloat16
AF = mybir.ActivationFunctionType

DIM = 48
HEADS = 4
DH = 16
WS = 800        # window (i dimension)
WSP = 896       # j dimension padded to 7*128
INNER = 64
SCALE = DH ** -0.5  # 0.25
B = 4
N = 6400
NW = 8
NCORES = 8
BLOCKS_PER_CORE = 4
NBI = 8  # block-inputs per core: 4 blocks x 2 inputs
NJ = 7   # j-chunks per window: 7x128 (padded)
NSTEP = NJ * HEADS  # 28 (jc, h) steps per block-input
ICHUNKS = [(0, 512), (512, 800)]    # i (token) chunks, <=512 for fp32 matmul
KCHUNKS = [(0, 512), (512, 896)]    # K^T free-dim chunks

# unit order: pair p = block-inputs (p, p+4) -> process adjacently
UNITS = [0, 4, 1, 5, 2, 6, 3, 7]

USE_TILE_POSITION = os.environ.get("KERNEL_NO_TILEPOS", "0") != "1"
# benchmarking: repeat the whole kernel body R times inside the NEFF so the
# marginal wall-clock per rep isolates device time from dispatch overhead
REPS = int(os.environ.get("KERNEL_REPS", "1"))
# DIAG="noexp": replace the ACT exp with nothing (O reads a constant tile)
# to measure the PE/DVE/DMA pipeline floor. Diagnostic only — wrong results.
DIAG = os.environ.get("KERNEL_DIAG", "")


def _tp(row, col):
    return (row, col) if USE_TILE_POSITION else None


class _Body:
    """Emission helper holding pools/constants for one kernel body."""

    def __init__(self, ctx, tc, y_ap, xt_ap, consts):
        self.nc = tc.nc
        self.tc = tc
        self.y_ap = y_ap
        self.xt_ap = xt_ap
        (self.wq_s, self.wk_s, self.wva_s, self.wouta_s,
         self.b2_s) = consts

        self.xt_pool = ctx.enter_context(tc.tile_pool(name="xt", bufs=2))
        self.qkt_pool = ctx.enter_context(tc.tile_pool(name="qkt", bufs=2))
        self.vaug_pool = ctx.enter_context(tc.tile_pool(name="vaug", bufs=9))
        self.e_pool = ctx.enter_context(tc.tile_pool(name="e", bufs=4))
        self.ots_pool = ctx.enter_context(tc.tile_pool(name="ots", bufs=3))
        self.den_pool = ctx.enter_context(tc.tile_pool(name="den", bufs=2))
        self.ont_pool = ctx.enter_context(tc.tile_pool(name="ont", bufs=3))
        self.rs_pool = ctx.enter_context(tc.tile_pool(name="rs", bufs=2))
        self.outb_pool = ctx.enter_context(tc.tile_pool(name="outb", bufs=2))

        self.ps_s = ctx.enter_context(
            tc.tile_pool(name="ps_s", bufs=2, space="PSUM"))
        self.ps_ot = ctx.enter_context(
            tc.tile_pool(name="ps_ot", bufs=1, space="PSUM"))
        self.ps_p = ctx.enter_context(
            tc.tile_pool(name="ps_p", bufs=2, space="PSUM"))

        # per-unit live state
        self.xts = {}     # bi -> xt tile
        self.qkt = {}     # bi -> qkt tile
        self.va = {}      # (bi, jc) -> V_aug tile
        self.sg = {}      # (bi, t) -> S^T psum tile
        self.ot = {}      # bi -> O^T psum accumulator
        self.ots = {}     # bi -> O^T sbuf copy
        self.den = {}     # pair -> denominator tile
        self.rcp = {}     # pair -> reciprocal tile
        self.onts = {}    # pair -> [ont0, ont1]
        self.e_const = None

    # ---- emission pieces ----
    def emit_xt_dma(self, bi):
        nc = self.nc
        xts = self.xt_pool.tile([49, WSP], BF16, tag="xt", name="xts")
        nc.sync.dma_start(xts[:], self.xt_ap[bi, :, :])
        self.xts[bi] = xts

    def emit_qkv(self, bi, which):
        """which: 0 emits Q^T projection, 1 emits K^T projection."""
        nc = self.nc
        if bi not in self.qkt:
            self.qkt[bi] = self.qkt_pool.tile([128, 2, WSP], BF16, tag="qkt", name="qkt")
        qkt = self.qkt[bi]
        xts = self.xts[bi]
        w_s = self.wq_s if which == 0 else self.wk_s
        chunks = ICHUNKS if which == 0 else KCHUNKS
        for n0, n1 in chunks:
            pp = self.ps_p.tile([128, 512], F32, tag="p", name="pp")
            nc.tensor.matmul(pp[:, 0:n1 - n0], w_s[:, :], xts[0:48, n0:n1],
                             start=True, stop=True)
            nc.vector.tensor_copy(qkt[:, which, n0:n1], pp[:, 0:n1 - n0])

    def emit_v(self, bi, jc):
        nc = self.nc
        j0 = jc * 128
        va = self.vaug_pool.tile([128, 128], BF16, tag="va", name="va")
        vp = self.ps_p.tile([128, 512], F32, tag="p", name="pp")
        nc.tensor.matmul(vp[:, 0:128], self.xts[bi][:, j0:j0 + 128],
                         self.wva_s[:, :], start=True, stop=True)
        nc.vector.tensor_copy(va[:, :], vp[:, 0:128])
        self.va[(bi, jc)] = va

    def emit_s(self, bi, t):
        nc = self.nc
        h, jc = t // NJ, t % NJ
        j0 = jc * 128
        qkt = self.qkt[bi]
        sg = self.ps_s.tile([128, 1024], F32, tag="sg", name="sg")
        for n0, n1 in ICHUNKS:
            nc.tensor.matmul(
                sg[:, n0:n1],
                qkt[32 * h:32 * h + 16, 1, j0:j0 + 128],
                qkt[32 * h:32 * h + 16, 0, n0:n1],
                start=True, stop=True,
                tile_position=_tp(32 * h, 0),
            )
        self.sg[(bi, t)] = sg

    def emit_exp(self, bi, t):
        nc = self.nc
        sg = self.sg.pop((bi, t))
        if DIAG == "noexp":
            if self.e_const is None:
                self.e_const = self.e_pool.tile([128, WS], BF16, tag="ec",
                                                name="ec")
                nc.vector.memset(self.e_const[:], 1.0)
            return self.e_const
        e = self.e_pool.tile([128, WS], BF16, tag="e", name="e")
        nc.scalar.activation(e[:, :], sg[:, 0:WS], AF.Exp, scale=float(SCALE))
        return e

    def emit_o(self, bi, t, e):
        nc = self.nc
        h, jc = t // NJ, t % NJ
        if bi not in self.ot:
            self.ot[bi] = self.ps_ot.tile([128, 1024], F32, tag="ot", name="ot")
        ot = self.ot[bi]
        va = self.va[(bi, jc)]
        for n0, n1 in ICHUNKS:
            nc.tensor.matmul(
                ot[32 * h:32 * h + 32, n0:n1],
                va[:, 32 * h:32 * h + 32],
                e[:, n0:n1],
                start=(jc == 0), stop=(jc == NJ - 1),
                tile_position=_tp(0, 32 * h),
            )
        if h == HEADS - 1:
            del self.va[(bi, jc)]
        return h, jc

    def emit_head_epilogue(self, bi, h, half, pair):
        """Head h's O^T strip is complete: copy it PSUM -> SBUF and gather
        its denominator row.  Spreading this through the unit (h-outer step
        order) leaves only head 3's chain on the kernel tail."""
        nc = self.nc
        ot = self.ot[bi] if h < HEADS - 1 else self.ot.pop(bi)
        if bi not in self.ots:
            self.ots[bi] = self.ots_pool.tile([128, WS], F32, tag="ots",
                                              name="ots")
        ots = self.ots[bi]
        nc.vector.tensor_copy(ots[32 * h:32 * h + 32, :],
                              ot[32 * h:32 * h + 32, 0:WS])
        if (pair, half) not in self.den:
            self.den[(pair, half)] = self.den_pool.tile([4, WS], F32,
                                                        tag="den", name="den")
        den = self.den[(pair, half)]
        nc.sync.dma_start(den[h:h + 1, :], ots[32 * h + 16:32 * h + 17, :])

    def emit_epilogue(self, bi):
        self.qkt.pop(bi, None)
        self.xts.pop(bi, None)

    def emit_recip_half(self, pair, half):
        """Reciprocal of one half's denominator rows (DVE), then broadcast
        them to the O^T partition layout with 8 parallel stride-0 DMAs.
        Running this at the owning unit's end hides the ~17us broadcast
        latency (128 x 1600B descriptors) under the next unit's compute."""
        nc = self.nc
        den = self.den.pop((pair, half))
        rcp = self.den_pool.tile([4, WS], F32, tag="rcp", name="rcp")
        scr = self.den_pool.tile([4, WS], F32, tag="scr", name="scr")
        nc.vector.reciprocal_approx_accurate(rcp[:], den[0:4, :], scr[:])
        rcpb = self.den_pool.tile([4, WS], BF16, tag="rcpb", name="rcpb")
        nc.vector.tensor_copy(rcpb[:], rcp[:])
        if (pair, half) == (3, 1):
            # kernel tail: broadcast via a tiny PE matmul (block-indicator
            # stationary) into PSUM - the DMA broadcast's issue+transfer
            # latency (~9us) would sit exposed on the critical path
            ps_rs = self.ps_s.tile([128, 1024], F32, tag="sg", name="ps_rs")
            for n0, n1 in ICHUNKS:
                nc.tensor.matmul(ps_rs[:, n0:n1], self.bsel_s[:, :],
                                 rcpb[:, n0:n1], start=True, stop=True)
            self.rcp[(pair, half)] = ps_rs
        else:
            rs = self.rs_pool.tile([128, WS], BF16, tag="rs", name="rs")
            for k in range(8):
                r = k // 2
                src = rcpb[r:r + 1, :].unsqueeze(1)
                nc.sync.dma_start(rs[16 * k:16 * k + 16, :],
                                  src.to_broadcast([1, 16, WS]))
            self.rcp[(pair, half)] = rs

    def emit_norm(self, pair):
        """Normalize on the DVE (broadcasts issued at each unit's end),
        then pre-sum the two halves so the projection runs half the
        matmuls."""
        nc = self.nc
        onts = []
        for half in range(2):
            bi = pair + 4 * half
            rs = self.rcp.pop((pair, half))
            ont = self.ont_pool.tile([128, WS], BF16, tag="ont", name="ont")
            if (pair, half) == (3, 1):
                nc.vector.tensor_mul(ont[:], self.ots.pop(bi)[:],
                                     rs[:, 0:WS])
            else:
                nc.vector.tensor_mul(ont[:], self.ots.pop(bi)[:], rs[:])
            onts.append(ont)
        osum = self.ont_pool.tile([128, WS], BF16, tag="osum", name="osum")
        nc.vector.tensor_add(osum[:], onts[0][:], onts[1][:])
        self.onts[pair] = osum

    def emit_outproj(self, pair):
        """out^T = Wout_all^T @ (ont0 + ont1): 4 matmuls sharing one LDW,
        bias via per-partition scalar add, transposed DMA to DRAM."""
        nc = self.nc
        osum = self.onts.pop(pair)
        outb = self.outb_pool.tile([48, WS], F32, tag="outb", name="outb")
        for n0, n1 in ICHUNKS:
            op = self.ps_p.tile([128, 512], F32, tag="p", name="pp")
            nc.tensor.matmul(op[0:48, 0:n1 - n0], self.wouta_s[:, :],
                             osum[:, n0:n1], start=True, stop=True)
            nc.vector.tensor_scalar_add(outb[:, n0:n1], op[0:48, 0:n1 - n0],
                                        self.b2_s[:, 0:1])
        nc.sync.dma_start(self.y_ap[pair], outb[:])


def build_kernel_body(ctx, tc, y_ap, xt_ap, wq_ap, wk_ap, wva_ap, wouta_ap,
                      b2_ap, bsel_ap):
    nc = tc.nc
    consts = ctx.enter_context(tc.tile_pool(name="consts", bufs=1))
    bsel_s = consts.tile([4, 128], BF16, tag="bsel")
    wq_s = consts.tile([48, 128], BF16, tag="wq")
    wk_s = consts.tile([48, 128], BF16, tag="wk")
    wva_s = consts.tile([49, 128], BF16, tag="wva")
    wouta_s = consts.tile([128, 48], BF16, tag="wouta")
    b2_s = consts.tile([48, 1], F32, tag="b2")

    body = _Body(ctx, tc, y_ap, xt_ap,
                 (wq_s, wk_s, wva_s, wouta_s, b2_s))
    body.bsel_s = bsel_s

    # the first unit's X^T load gates the whole pipeline: put it at the
    # head of the DMA queue, before the constants (wq next - the first
    # projection needs it; bsel last - only the kernel tail reads it)
    body.emit_xt_dma(UNITS[0])
    nc.sync.dma_start(wq_s[:], wq_ap[:, :])
    nc.sync.dma_start(wk_s[:], wk_ap[:, :])
    nc.sync.dma_start(wva_s[:], wva_ap[:, :])
    nc.sync.dma_start(wouta_s[:], wouta_ap[:, :])
    nc.sync.dma_start(b2_s[:], b2_ap[:, :])
    nc.sync.dma_start(bsel_s[:], bsel_ap[:, :])

    for _rep in range(REPS):
        _emit_pipeline(body)


def _emit_pipeline(body):
    """Flattened, software-pipelined emission over all 8 block-input units."""
    # prologue for the first unit (xt DMA already issued at queue head)
    bi0 = UNITS[0]
    if bi0 not in body.xts:
        body.emit_xt_dma(bi0)
    body.emit_qkv(bi0, 0)
    body.emit_qkv(bi0, 1)
    body.emit_v(bi0, 0)
    body.emit_s(bi0, 0)

    for u, bi in enumerate(UNITS):
        nxt = UNITS[u + 1] if u + 1 < len(UNITS) else None
        for t in range(NSTEP):
            # hoisted prologue work for the next unit, placed in the exp
            # shadow near the end of this unit
            if nxt is not None:
                if t == 20:
                    body.emit_xt_dma(nxt)
                elif t == 22:
                    body.emit_qkv(nxt, 0)
                elif t == 24:
                    body.emit_qkv(nxt, 1)
            # deferred normalization of the previous pair, injected early in
            # this unit so its PE/DVE work hides under this unit's exp stream
            if u % 2 == 0 and u > 0:
                if t == 2:
                    body.emit_norm(u // 2 - 1)
                elif t == 8:
                    body.emit_outproj(u // 2 - 1)
            # emit S one step ahead of O so the in-order PE queue never has
            # an un-issued S behind a stalled O
            if 1 <= t + 1 <= 6:
                body.emit_v(bi, t + 1)
            if t + 1 < NSTEP:
                body.emit_s(bi, t + 1)
            elif nxt is not None:
                body.emit_v(nxt, 0)
                body.emit_s(nxt, 0)
            e = body.emit_exp(bi, t)
            h, jc = body.emit_o(bi, t, e)
            if jc == NJ - 1:
                body.emit_head_epilogue(bi, h, half=u % 2, pair=u // 2)
                if h == HEADS - 1:
                    body.emit_recip_half(u // 2, u % 2)
        body.emit_epilogue(bi)

    # final pair's normalization runs in the tail
    last_pair = len(UNITS) // 2 - 1
    body.emit_norm(last_pair)
    body.emit_outproj(last_pair)


def _dedup_ldweights(nc):
    """Drop InstLdweights that reload the exact weights the PE already holds.

    tile_legalize splits every non-f32 matmul into LDW+MM; chunked matmuls
    sharing one stationary operand then carry a redundant second LDW (no
    waits/updates).  Removing it saves ~110ns of PE sequencer time each."""
    fn = nc.m.functions[0]
    for bb in fn.blocks:
        insts = bb.instructions
        drop = []
        prev_key = None
        for k in range(len(insts)):
            inst = insts[k]
            tn = type(inst).__name__
            if str(inst.engine) != "EngineType.PE":
                continue
            if tn == "InstLdweights":
                w = inst.ins[0]
                key = (w.memsetref, w.offset, str(w.ap), str(w.dtype),
                       str(inst.tile_position), str(inst.tile_size),
                       str(inst.perf_mode))
                si = inst.sync_info
                clean = si is None or (not si.on_wait and not si.on_update)
                if key == prev_key and clean:
                    drop.append(k)
                    continue
                prev_key = key
            elif tn == "InstMatmult":
                pass  # same weights keep streaming; array not clobbered
            else:
                prev_key = None
        for k in reversed(drop):
            del insts[k]


_CACHED = {}


def build_nc():
    key = (REPS, USE_TILE_POSITION, DIAG)
    if key in _CACHED:
        return _CACHED[key]
    nc = bacc.Bacc("TRN2", target_bir_lowering=False, debug=False)
    xt = nc.dram_tensor("xt", [NBI, 49, WSP], BF16, kind="ExternalInput")
    wq = nc.dram_tensor("wq", [48, 128], BF16, kind="ExternalInput")
    wk = nc.dram_tensor("wk", [48, 128], BF16, kind="ExternalInput")
    wva = nc.dram_tensor("wva", [49, 128], BF16, kind="ExternalInput")
    wouta = nc.dram_tensor("wouta", [128, 48], BF16, kind="ExternalInput")
    b2 = nc.dram_tensor("b2", [48, 1], F32, kind="ExternalInput")
    bsel = nc.dram_tensor("bsel", [4, 128], BF16, kind="ExternalInput")
    y = nc.dram_tensor("y", [BLOCKS_PER_CORE, DIM, WS], F32,
                       kind="ExternalOutput")
    with tile.TileContext(nc) as tc:
        with ExitStack() as ctx:
            build_kernel_body(ctx, tc, y.ap(), xt.ap(), wq.ap(), wk.ap(),
                              wva.ap(), wouta.ap(), b2.ap(), bsel.ap())
    nc.compile()
    _dedup_ldweights(nc)
    _CACHED[key] = nc
    return nc


def _prep_consts(Wqkv, Wout, bout):
    WQ = np.zeros((48, 128), np.float32)
    WK = np.zeros((48, 128), np.float32)
    WVA = np.zeros((49, 128), np.float32)
    WOUTA = np.zeros((128, 48), np.float32)
    for h in range(HEADS):
        WQ[:, 32 * h:32 * h + 16] = Wqkv[h * 48:h * 48 + 16, :].T
        WK[:, 32 * h:32 * h + 16] = Wqkv[h * 48 + 16:h * 48 + 32, :].T
        WVA[0:48, 32 * h:32 * h + 16] = Wqkv[h * 48 + 32:h * 48 + 48, :].T
        WVA[48, 32 * h + 16] = 1.0
        WOUTA[32 * h:32 * h + 16, :] = Wout[:, 16 * h:16 * h + 16].T
    B2 = (2.0 * bout).astype(np.float32)[:, None]
    BSEL = np.zeros((4, 128), np.float32)
    for r in range(4):
        BSEL[r, 32 * r:32 * r + 32] = 1.0
    BF = ml_dtypes.bfloat16
    return (WQ.astype(BF), WK.astype(BF), WVA.astype(BF), WOUTA.astype(BF),
            B2, BSEL.astype(BF))


def kernel(x1, x2, Wqkv, Wout, bout):
    x1 = np.ascontiguousarray(x1, np.float32)
    x2 = np.ascontiguousarray(x2, np.float32)
    Wqkv = np.asarray(Wqkv, np.float32)
    Wout = np.asarray(Wout, np.float32)
    bout = np.asarray(bout, np.float32)

    nc = build_nc()
    WQ, WK, WVA, WOUTA, B2, BSEL = _prep_consts(Wqkv, Wout, bout)

    in_maps = []
    for c in range(NCORES):
        XT = np.zeros((NBI, 49, WSP), ml_dtypes.bfloat16)
        for j in range(BLOCKS_PER_CORE):
            g = 4 * c + j
            b, w = g // NW, g % NW
            XT[j, 0:48, 0:WS] = x1[b, w * WS:(w + 1) * WS, :].T
            XT[j, 48, 0:WS] = 1.0
            XT[4 + j, 0:48, 0:WS] = x2[b, w * WS:(w + 1) * WS, :].T
            XT[4 + j, 48, 0:WS] = 1.0
        in_maps.append({
            "xt": XT, "wq": WQ, "wk": WK, "wva": WVA,
            "wouta": WOUTA, "b2": B2, "bsel": BSEL,
        })

    res = run_bass_kernel_spmd(nc, in_maps, core_ids=list(range(NCORES)))
    kernel._last_results = res

    out = np.empty((B, N, DIM), np.float32)
    for c in range(NCORES):
        y = res.results[c]["y"]
        for j in range(BLOCKS_PER_CORE):
            g = 4 * c + j
            b, w = g // NW, g % NW
            out[b, w * WS:(w + 1) * WS, :] = y[j].T
    return out

